# revision 1
# baseline (speedup 1.0000x reference)
"""Multi-head attention (B=1, S=2048, D=2048, H=16, d_k=128) on 8 Trainium2
NeuronCores via Bass/Tile.

Sharding: tensor-parallel over heads. Each core owns 2 heads: it gets the
column shards of Wq/Wk/Wv and the row shard of Wo for those heads, computes
its partial output projection, and the host sums the 8 partials (the
all-reduce equivalent) and adds biases.

All matmuls run in fp16 (single-pass, full PE rate; fp32 PSUM
accumulation). Measured end-to-end relative error ~1e-3 against the fp32
reference, dominated by fp16 rounding of x/W/Q/K.

Per-core dataflow (everything derived from x^T so contractions sit on the
partition axis):
  phase 1: Q^T = Wq_s.T @ x^T-chunks, K^T likewise, V = x @ Wv_s (natural)
  phase 2: per (head, q-chunk): S^T[k,q] = K^T.T @ Q^T (one 128-contraction
           matmul per k-tile), ACT exp -> P^T (f32r), then two accumulating
           matmuls over the 16 k-tiles: ones-lhsT -> softmax denominator
           (broadcast across partitions) and V-lhsT -> unnormalized O^T.
           DVE reciprocal+multiply normalizes.
  phase 3: out_partial[q,e] = O^T.T @ Wo_s, accumulated over the 2 d-tiles.
"""

import sys

sys.path.insert(0, "/opt/trn_rl_repo")

import numpy as np

S = 2048
D = 2048
H = 16
DK = 128
N_CORES = 8
HEADS_PER_CORE = H // N_CORES  # 2
DPC = HEADS_PER_CORE * DK  # 256, per-core projection width
SCALE = 1.0 / np.sqrt(DK)

TRACE = False  # test.py flips this to get an NTFF profile + exec time
_LAST_EXEC_NS = [None]
_LAST_RESULTS = [None]

_PROGRAM = [None]


def _build_program():
    from concourse import bacc, bass_isa, mybir
    from concourse.tile import TileContext

    f32 = mybir.dt.float32
    f32r = mybir.dt.float32r
    f16 = mybir.dt.float16

    nc = bacc.Bacc()
    xT = nc.declare_dram_parameter("xT", [D, S], f16, isOutput=False)
    wq = nc.declare_dram_parameter("wq", [D, DPC], f16, isOutput=False)
    wk = nc.declare_dram_parameter("wk", [D, DPC], f16, isOutput=False)
    wv = nc.declare_dram_parameter("wv", [D, DPC], f16, isOutput=False)
    wo = nc.declare_dram_parameter("wo", [DPC, S], f16, isOutput=False)
    out = nc.declare_dram_parameter("out", [S, D], f32, isOutput=True)

    ND = D // 128  # 16 d-tiles of the model dim
    NS = S // 128  # 16 s-tiles
    NQ = S // 512  # 4 q/s chunks
    EXP = mybir.ActivationFunctionType.Exp
    CPY = mybir.ActivationFunctionType.Copy

    xT_r = xT[:].rearrange("(t p) s -> p t s", p=128)  # [128, 16, 2048]
    wq_r = wq[:].rearrange("(t p) n -> p t n", p=128)  # [128, 16, 256]
    wk_r = wk[:].rearrange("(t p) n -> p t n", p=128)
    wv_r = wv[:].rearrange("(t p) n -> p t n", p=128)
    wo_r = wo[:].rearrange("(t p) e -> p t e", p=128)  # [128, 2, 2048]

    with TileContext(nc) as tc:
        with (
            tc.tile_pool(name="wpool", bufs=1) as wpool,
            tc.tile_pool(name="xpool", bufs=3) as xpool,
            tc.tile_pool(name="qkv", bufs=1) as qkv,
            tc.tile_pool(name="ppool", bufs=3) as ppool,
            tc.tile_pool(name="rpool", bufs=1) as rpool,
            tc.tile_pool(name="obpool", bufs=3) as obpool,
            tc.tile_pool(name="psA", bufs=2, space="PSUM") as psA,
            tc.tile_pool(name="psB", bufs=2, space="PSUM") as psB,
            tc.tile_pool(name="psC", bufs=2, space="PSUM") as psC,
        ):
            # resident weights (qkv projections); wo rides in an xpool slot
            # after the x^T stream is done with it.
            wq_sb = wpool.tile([128, ND * DPC], f16, tag="wq")
            wk_sb = wpool.tile([128, ND * DPC], f16, tag="wk")
            wv_sb = wpool.tile([128, ND * DPC], f16, tag="wv")

            # per-head Q^T/K^T [128, S] and V in natural layout [128, NS*DPC]
            qt_sb = [qkv.tile([128, S], f16, tag=f"qt{h}", name=f"qt{h}") for h in range(2)]
            kt_sb = [qkv.tile([128, S], f16, tag=f"kt{h}", name=f"kt{h}") for h in range(2)]
            v_sb = qkv.tile([128, NS * DPC], f16, tag="v")

            # ---------------- phase 1: projections ----------------
            for sc in range(NQ):  # 4 chunks of 512 seq positions
                xt = xpool.tile([128, ND * 512], f16, tag="xt")
                if sc == 0:
                    # interleaved per-d-tile DMAs: the d-tile-i matmul of the
                    # first Q^T/K^T accumulation only waits for slice i, so PE
                    # starts ~2us in instead of after the full 8MB prefetch
                    for dt_ in range(ND):
                        nc.sync.dma_start(
                            out=wq_sb[:, dt_ * DPC:(dt_ + 1) * DPC],
                            in_=wq_r[:, dt_, :],
                        )
                        nc.sync.dma_start(
                            out=xt[:, dt_ * 512:(dt_ + 1) * 512],
                            in_=xT_r[:, dt_, 0:512],
                        )
                    for dt_ in range(ND):
                        nc.sync.dma_start(
                            out=wk_sb[:, dt_ * DPC:(dt_ + 1) * DPC],
                            in_=wk_r[:, dt_, :],
                        )
                else:
                    for dt_ in range(ND):
                        nc.sync.dma_start(
                            out=xt[:, dt_ * 512:(dt_ + 1) * 512],
                            in_=xT_r[:, dt_, sc * 512:(sc + 1) * 512],
                        )
                # Q^T and K^T: [n_tile 128, s 512] = sum_d W[d, n].T @ xT[d, s]
                for w_sb, dst in ((wq_sb, qt_sb), (wk_sb, kt_sb)):
                    for h in range(2):
                        ps = psA.tile([128, 512], f32, tag="proj")
                        for dt_ in range(ND):
                            nc.tensor.matmul(
                                ps[:],
                                w_sb[:, dt_ * DPC + h * 128: dt_ * DPC + h * 128 + 128],
                                xt[:, dt_ * 512:(dt_ + 1) * 512],
                                start=(dt_ == 0),
                                stop=(dt_ == ND - 1),
                            )
                        nc.vector.tensor_copy(
                            dst[h][:, sc * 512:(sc + 1) * 512], ps[:]
                        )
                    if sc == 0 and w_sb is wq_sb:
                        # wv arrives while the first K^T chunk computes
                        nc.sync.dma_start(
                            out=wv_sb[:].rearrange("p (t n) -> p t n", n=DPC),
                            in_=wv_r,
                        )
                # V natural: [s_tile 128, 256] = sum_d xT[d, s_tile].T @ Wv[d, :]
                for st in range(4):
                    s_tile = sc * 4 + st
                    ps = psA.tile([128, 512], f32, tag="proj")
                    for dt_ in range(ND):
                        nc.tensor.matmul(
                            ps[:, 0:DPC],
                            xt[:, dt_ * 512 + st * 128: dt_ * 512 + st * 128 + 128],
                            wv_sb[:, dt_ * DPC:(dt_ + 1) * DPC],
                            start=(dt_ == 0),
                            stop=(dt_ == ND - 1),
                        )
                    nc.vector.tensor_copy(
                        v_sb[:, s_tile * DPC:(s_tile + 1) * DPC], ps[:, 0:DPC]
                    )

            # wo reuses a retired x^T-stream slot (same tag/shape)
            wo_ot = xpool.tile([128, ND * 512], f16, tag="xt")
            wo_sb = wo_ot[:, 0:2 * S]
            nc.sync.dma_start(
                out=wo_sb.rearrange("p (t e) -> p t e", e=S), in_=wo_r
            )

            # ------- phases 2+3 interleaved per q-chunk -------
            # attention for both heads of a q-chunk; the chunk's
            # output-projection rows are emitted one chunk later so they
            # overlap the next chunk's attention instead of stalling on the
            # O^T normalize at the chunk boundary
            # O^T normalized output, one [128, 512] tile per (head, q-chunk);
            # separate tiles (not slices of one tensor) so the delayed
            # projection reads don't pick up false deps on later chunks'
            # writes. 6 slots: 2 chunks * 2 heads live + write-ahead.
            ot_tiles = {}

            def emit_proj(qc_done):
                for qt_ in range(qc_done * 4, qc_done * 4 + 4):
                    for ec in range(NQ):
                        ps = psA.tile([128, 512], f32, tag="proj", name="proj_ps")
                        for dt_ in range(2):
                            nc.tensor.matmul(
                                ps[:],
                                ot_tiles[(dt_, qc_done)][:, (qt_ - qc_done * 4) * 128:(qt_ - qc_done * 4 + 1) * 128],
                                wo_sb[:, dt_ * S + ec * 512: dt_ * S + ec * 512 + 512],
                                start=(dt_ == 0),
                                stop=(dt_ == 1),
                            )
                        ob = obpool.tile([128, 512], f32, tag="ob", name="ob")
                        if ec % 2 == 0:
                            nc.scalar.activation(ob[:], ps[:], CPY)
                        else:
                            nc.vector.tensor_copy(ob[:], ps[:])
                        nc.sync.dma_start(
                            out=out[qt_ * 128:(qt_ + 1) * 128, ec * 512:(ec + 1) * 512],
                            in_=ob[:],
                        )

            for qc in range(NQ):
                for h in range(2):
                    if h == 1 and qc > 1:
                        emit_proj(qc - 2)
                    oT = psB.tile([128, 512], f32, tag="oT", bufs=3, name="oT")
                    qt_slice = qt_sb[h][:, qc * 512:(qc + 1) * 512]
                    st_tiles = {}
                    st_tiles[0] = psC.tile([128, 512], f32, tag="st", name="st0", bufs=3)
                    nc.tensor.matmul(
                        st_tiles[0][:], kt_sb[h][:, 0:128], qt_slice,
                        start=True, stop=True,
                    )
                    # softmax denominator: DVE-accumulate the exp tiles, then
                    # one GPSIMD cross-partition all-reduce (broadcast result)
                    acc = rpool.tile([128, 512], f32, tag="acc", bufs=2, name="acc")
                    for kt_ in range(NS):
                        pt = ppool.tile([128, 512], f16, tag="pt")
                        nc.scalar.activation(
                            pt[:], st_tiles.pop(kt_)[:], EXP, scale=float(SCALE)
                        )
                        if kt_ + 1 < NS:
                            st_tiles[kt_ + 1] = psC.tile([128, 512], f32, tag="st", name="stn", bufs=3)
                            nc.tensor.matmul(
                                st_tiles[kt_ + 1][:],
                                kt_sb[h][:, (kt_ + 1) * 128:(kt_ + 2) * 128],
                                qt_slice,
                                start=True, stop=True,
                            )
                        if kt_ == 0:
                            nc.vector.tensor_copy(acc[:], pt[:])
                        else:
                            nc.vector.tensor_add(acc[:], acc[:], pt[:])
                        nc.tensor.matmul(
                            oT[:],
                            v_sb[:, kt_ * DPC + h * 128: kt_ * DPC + h * 128 + 128],
                            pt[:],
                            start=(kt_ == 0), stop=(kt_ == NS - 1),
                        )
                    bc = rpool.tile([128, 512], f32, tag="bc", bufs=2, name="bc")
                    nc.gpsimd.partition_all_reduce(
                        bc[:], acc[:], 128, bass_isa.ReduceOp.add
                    )
                    rc = rpool.tile([128, 512], f32, tag="recip")
                    # ~18-bit 1/x, ~5x faster than exact reciprocal; softmax
                    # denominators are well-conditioned positives (~1e2..4e3)
                    nc.vector.reciprocal_approx_fast(rc[:], bc[:])
                    ot_tiles[(h, qc)] = ppool.tile(
                        [128, 512], f16, tag="ot", bufs=8, name="ot_t"
                    )
                    nc.vector.tensor_mul(ot_tiles[(h, qc)][:], oT[:], rc[:])
            emit_proj(NQ - 2)
            emit_proj(NQ - 1)

    nc.compile()
    return nc


def _numpy_fallback(x, mask, Wq, bq, Wk, bk, Wv, bv, Wo, bo):
    B, S_, D_ = x.shape
    xf = x.reshape(S_, D_).astype(np.float64)

    def proj(W, b):
        y = xf @ W.astype(np.float64) + b.astype(np.float64)
        return y.reshape(S_, H, DK).transpose(1, 0, 2)

    Q = proj(Wq, bq)
    K = proj(Wk, bk)
    V = proj(Wv, bv)
    m = np.broadcast_to(mask, (B, H, S_, S_))
    out = np.empty((H, S_, DK))
    for h in range(H):
        sc = Q[h] @ K[h].T / np.sqrt(DK)
        sc = np.where(m[0, h], sc, -np.inf)
        sc -= sc.max(axis=-1, keepdims=True)
        e = np.exp(sc)
        p = e / e.sum(axis=-1, keepdims=True)
        out[h] = p @ V[h]
    o = out.transpose(1, 0, 2).reshape(S_, D_)
    res = o @ Wo.astype(np.float64) + bo.astype(np.float64)
    return res.reshape(B, S_, D_).astype(np.float32)


def kernel(x, mask, Wq, bq, Wk, bk, Wv, bv, Wo, bo):
    x = np.asarray(x, dtype=np.float32)
    mask = np.asarray(mask)
    Wq = np.asarray(Wq, dtype=np.float32)
    Wk = np.asarray(Wk, dtype=np.float32)
    Wv = np.asarray(Wv, dtype=np.float32)
    Wo = np.asarray(Wo, dtype=np.float32)
    bq = np.asarray(bq, dtype=np.float32)
    bk = np.asarray(bk, dtype=np.float32)
    bv = np.asarray(bv, dtype=np.float32)
    bo = np.asarray(bo, dtype=np.float32)

    # Off-benchmark shapes/masks/biases: exact numpy fallback.
    # (bk shifts every score row by a constant -> softmax-invariant; bv and bo
    # are affine in the output and folded in on the host; only bq actually
    # changes the attention pattern in a way the device kernel doesn't model.)
    if x.shape != (1, S, D) or not bool(mask.all()) or np.any(bq):
        return _numpy_fallback(x, mask, Wq, bq, Wk, bk, Wv, bv, Wo, bo)

    from concourse.bass_utils import run_bass_kernel_spmd

    if _PROGRAM[0] is None:
        _PROGRAM[0] = _build_program()
    nc = _PROGRAM[0]

    xT = np.ascontiguousarray(x.reshape(S, D).T.astype(np.float16))
    in_maps = []
    for c in range(N_CORES):
        lo, hi = c * DPC, (c + 1) * DPC
        in_maps.append(
            {
                "xT": xT,
                "wq": np.ascontiguousarray(Wq[:, lo:hi].astype(np.float16)),
                "wk": np.ascontiguousarray(Wk[:, lo:hi].astype(np.float16)),
                "wv": np.ascontiguousarray(Wv[:, lo:hi].astype(np.float16)),
                "wo": np.ascontiguousarray(Wo[lo:hi, :].astype(np.float16)),
            }
        )

    res = run_bass_kernel_spmd(nc, in_maps, list(range(N_CORES)), trace=TRACE)
    _LAST_EXEC_NS[0] = res.exec_time_ns
    _LAST_RESULTS[0] = res

    acc = res.results[0]["out"].astype(np.float64)
    for c in range(1, N_CORES):
        acc += res.results[c]["out"]
    # bv contributes (attn rows sum to 1) a constant bv @ Wo; bo is additive.
    acc += (bv.astype(np.float64) @ Wo) + bo
    return acc.astype(np.float32).reshape(1, S, D)



# revision 3
# speedup vs baseline: 1.0831x; 1.0831x over previous
"""Multi-head attention (B=1, S=2048, D=2048, H=16, d_k=128) on 8 Trainium2
NeuronCores via Bass/Tile.

Sharding: tensor-parallel over heads. Each core owns 2 heads: it gets the
column shards of Wq/Wk/Wv and the row shard of Wo for those heads, computes
its partial output projection (in f16), and the host sums the 8 partials
(the all-reduce equivalent) and adds biases.

All matmuls run in fp16 (single-pass, full PE rate; fp32 PSUM accumulation).

v2 schedule: the kernel is ordered so the softmax pointwise work (ACT exp,
DVE denominator adds) overlaps the projection matmuls instead of running
after them:
  A: K^T projection for all 4 seq chunks (both heads) while x^T streams in.
  B: Q^T projection for chunk 0.
  C: scores+exp+denominator for (h, qc=0) emitted BEFORE the V projection
     and Q chunks 1-3, so ACT/DVE start ~35us in while PE continues with
     V/Q matmuls (~75us of independent PE work).
  D: P@V for qc=0 once V lands, then steady state per q-chunk: scores ->
     wide exp -> f16 denominator adds -> P@V, with the output projection of
     the previous chunk interleaved.

Pointwise cost structure vs v1:
  - exp runs on 2-PSUM-bank [128,1024] tiles (halves ACT per-inst overhead)
  - denominator accumulates in f16 wide tiles (DVE 2x mode) instead of f32
  - output partials are written/DMA'd as f16 (halves output DMA)
  - cross-partition denominator reduce stays on (otherwise idle) GPSIMD
"""

import sys

sys.path.insert(0, "/opt/trn_rl_repo")

import numpy as np

S = 2048
D = 2048
H = 16
DK = 128
N_CORES = 8
HEADS_PER_CORE = H // N_CORES  # 2
DPC = HEADS_PER_CORE * DK  # 256, per-core projection width
SCALE = 1.0 / np.sqrt(DK)

TRACE = False  # test.py flips this to get an NTFF profile + exec time
_LAST_EXEC_NS = [None]
_LAST_RESULTS = [None]

_PROGRAM = [None]


def _build_program():
    from concourse import bacc, bass_isa, mybir
    from concourse.tile import TileContext

    f32 = mybir.dt.float32
    f16 = mybir.dt.float16

    nc = bacc.Bacc()
    xT = nc.declare_dram_parameter("xT", [D, S], f16, isOutput=False)
    wq = nc.declare_dram_parameter("wq", [D, DPC], f16, isOutput=False)
    wk = nc.declare_dram_parameter("wk", [D, DPC], f16, isOutput=False)
    wv = nc.declare_dram_parameter("wv", [D, DPC], f16, isOutput=False)
    wo = nc.declare_dram_parameter("wo", [DPC, S], f16, isOutput=False)
    out = nc.declare_dram_parameter("out", [S, D], f16, isOutput=True)

    ND = D // 128  # 16 d-tiles of the model dim
    NS = S // 128  # 16 k-tiles
    NQ = S // 512  # 4 q chunks
    EXP = mybir.ActivationFunctionType.Exp

    xT_r = xT[:].rearrange("(t p) s -> p t s", p=128)  # [128, 16, 2048]
    wq_r = wq[:].rearrange("(t p) n -> p t n", p=128)  # [128, 16, 256]
    wk_r = wk[:].rearrange("(t p) n -> p t n", p=128)
    wv_r = wv[:].rearrange("(t p) n -> p t n", p=128)
    wo_r = wo[:].rearrange("(t p) e -> p t e", p=128)  # [128, 2, 2048]

    with TileContext(nc) as tc:
        with (
            tc.tile_pool(name="wpool", bufs=1) as wpool,
            tc.tile_pool(name="xpool", bufs=4) as xpool,
            tc.tile_pool(name="qkv", bufs=1) as qkv,
            tc.tile_pool(name="ppool", bufs=18) as ppool,
            tc.tile_pool(name="apool", bufs=2) as apool,
            tc.tile_pool(name="otpool", bufs=6) as otpool,
            tc.tile_pool(name="obpool", bufs=6) as obpool,
            tc.tile_pool(name="psP", bufs=2, space="PSUM") as psP,
            tc.tile_pool(name="psS", bufs=2, space="PSUM") as psS,
            tc.tile_pool(name="psO", bufs=2, space="PSUM") as psO,
        ):
            wq_sb = wpool.tile([128, ND * DPC], f16, tag="wq")
            wk_sb = wpool.tile([128, ND * DPC], f16, tag="wk")
            wv_sb = wpool.tile([128, ND * DPC], f16, tag="wv")
            wo_sb = wpool.tile([128, 2 * S], f16, tag="wo")

            # per-head Q^T/K^T [128 dk, S] and V natural [128 k, NS*DPC]
            qt_sb = [qkv.tile([128, S], f16, tag=f"qt{h}", name=f"qt{h}") for h in range(2)]
            kt_sb = [qkv.tile([128, S], f16, tag=f"kt{h}", name=f"kt{h}") for h in range(2)]
            v_sb = qkv.tile([128, NS * DPC], f16, tag="v")

            # ---- phase A: x^T stream + K^T projection (all chunks) ----
            xt_tiles = {}
            for sc in range(NQ):
                xt = xpool.tile([128, ND * 512], f16, tag="xt", name="xt_t")
                xt_tiles[sc] = xt
                for dt_ in range(ND):
                    # interleave one weight tensor's d-tiles with each x
                    # chunk so the first-needed weights land first
                    if sc == 0:
                        nc.sync.dma_start(
                            out=wk_sb[:, dt_ * DPC:(dt_ + 1) * DPC],
                            in_=wk_r[:, dt_, :],
                        )
                    elif sc == 1:
                        nc.sync.dma_start(
                            out=wq_sb[:, dt_ * DPC:(dt_ + 1) * DPC],
                            in_=wq_r[:, dt_, :],
                        )
                    elif sc == 2:
                        nc.sync.dma_start(
                            out=wv_sb[:, dt_ * DPC:(dt_ + 1) * DPC],
                            in_=wv_r[:, dt_, :],
                        )
                    nc.sync.dma_start(
                        out=xt[:, dt_ * 512:(dt_ + 1) * 512],
                        in_=xT_r[:, dt_, sc * 512:(sc + 1) * 512],
                    )
                for h in range(2):
                    ps = psP.tile([128, 512], f32, tag="proj", name="proj_ps")
                    for dt_ in range(ND):
                        nc.tensor.matmul(
                            ps[:],
                            wk_sb[:, dt_ * DPC + h * 128: dt_ * DPC + h * 128 + 128],
                            xt[:, dt_ * 512:(dt_ + 1) * 512],
                            start=(dt_ == 0),
                            stop=(dt_ == ND - 1),
                        )
                    nc.any.tensor_copy(kt_sb[h][:, sc * 512:(sc + 1) * 512], ps[:])

            # ---- phase B: Q^T projection, chunk 0 ----
            for h in range(2):
                ps = psP.tile([128, 512], f32, tag="proj", name="proj_ps")
                for dt_ in range(ND):
                    nc.tensor.matmul(
                        ps[:],
                        wq_sb[:, dt_ * DPC + h * 128: dt_ * DPC + h * 128 + 128],
                        xt_tiles[0][:, dt_ * 512:(dt_ + 1) * 512],
                        start=(dt_ == 0),
                        stop=(dt_ == ND - 1),
                    )
                nc.any.tensor_copy(qt_sb[h][:, 0:512], ps[:])
            nc.sync.dma_start(
                out=wo_sb[:].rearrange("p (t e) -> p t e", e=S), in_=wo_r
            )

            # ---- attention helpers ----
            # scores -> wide exp -> f16 denominator accumulation for (h, qc).
            # Returns (pt tiles, broadcast denominator handle is deferred).
            den_state = {}

            def attn_scores(h, qc):
                qt_slice = qt_sb[h][:, qc * 512:(qc + 1) * 512]
                pts = []
                acc = None
                for g in range(8):
                    st = psS.tile([128, 1024], f32, tag="st", name="st_ps")
                    for j in range(2):
                        kt = 2 * g + j
                        nc.tensor.matmul(
                            st[:, j * 512:(j + 1) * 512],
                            kt_sb[h][:, kt * 128:(kt + 1) * 128],
                            qt_slice,
                            start=True, stop=True,
                        )
                    pt = ppool.tile([128, 1024], f16, tag="pt", name="pt_t")
                    nc.scalar.activation(pt[:], st[:], EXP, scale=float(SCALE))
                    pts.append(pt)
                    if g == 1:
                        acc = apool.tile([128, 1024], f16, tag="acc", name="acc_t")
                        nc.vector.tensor_add(acc[:], pts[0][:], pts[1][:])
                    elif g > 1:
                        nc.vector.tensor_add(acc[:], acc[:], pt[:])
                return pts, acc

            def den_finalize(h, qc, acc):
                # fold the two 512-wide halves, cross-partition reduce on
                # GPSIMD (broadcasts), fast reciprocal
                accf = apool.tile([128, 512], f16, tag="accf", name="accf_t")
                nc.vector.tensor_add(accf[:], acc[:, 0:512], acc[:, 512:1024])
                bc = apool.tile([128, 512], f32, tag="bc", name="bc_t")
                nc.gpsimd.partition_all_reduce(
                    bc[:], accf[:], 128, bass_isa.ReduceOp.add
                )
                rc = apool.tile([128, 512], f32, tag="rc", name="rc_t")
                nc.vector.reciprocal_approx_fast(rc[:], bc[:])
                den_state[(h, qc)] = rc

            def pv(h, qc, pts):
                oT = psO.tile([128, 512], f32, tag="oT", name="oT_ps")
                for kt in range(NS):
                    pt = pts[kt // 2]
                    half = kt % 2
                    nc.tensor.matmul(
                        oT[:],
                        v_sb[:, kt * DPC + h * 128: kt * DPC + h * 128 + 128],
                        pt[:, half * 512:(half + 1) * 512],
                        start=(kt == 0), stop=(kt == NS - 1),
                    )
                return oT

            ot_tiles = {}

            def normalize(h, qc, oT):
                rc = den_state.pop((h, qc))
                ot = otpool.tile([128, 512], f16, tag="ot", name="ot_t")
                nc.vector.tensor_mul(ot[:], oT[:], rc[:])
                ot_tiles[(h, qc)] = ot

            def emit_oproj(qc):
                for qt_ in range(4):
                    for ec in range(NQ):
                        ps = psP.tile([128, 512], f32, tag="proj", name="proj_ps")
                        for h in range(2):
                            nc.tensor.matmul(
                                ps[:],
                                ot_tiles[(h, qc)][:, qt_ * 128:(qt_ + 1) * 128],
                                wo_sb[:, h * S + ec * 512: h * S + ec * 512 + 512],
                                start=(h == 0),
                                stop=(h == 1),
                            )
                        ob = obpool.tile([128, 512], f16, tag="ob", name="ob_t")
                        nc.any.tensor_copy(ob[:], ps[:])
                        nc.sync.dma_start(
                            out=out[(qc * 4 + qt_) * 128:(qc * 4 + qt_ + 1) * 128,
                                    ec * 512:(ec + 1) * 512],
                            in_=ob[:],
                        )

            # ---- phase C: qc=0 softmax pointwise overlaps V + Q1-3 ----
            pts00, acc00 = attn_scores(0, 0)
            pts10, acc10 = attn_scores(1, 0)
            den_finalize(0, 0, acc00)
            den_finalize(1, 0, acc10)

            # V projection (natural layout), all 16 k-tiles
            for sc in range(NQ):
                for st_ in range(4):
                    s_tile = sc * 4 + st_
                    ps = psP.tile([128, 512], f32, tag="proj", name="proj_ps")
                    for dt_ in range(ND):
                        nc.tensor.matmul(
                            ps[:, 0:DPC],
                            xt_tiles[sc][:, dt_ * 512 + st_ * 128: dt_ * 512 + st_ * 128 + 128],
                            wv_sb[:, dt_ * DPC:(dt_ + 1) * DPC],
                            start=(dt_ == 0),
                            stop=(dt_ == ND - 1),
                        )
                    nc.any.tensor_copy(
                        v_sb[:, s_tile * DPC:(s_tile + 1) * DPC], ps[:, 0:DPC]
                    )
            # Q^T projection, chunks 1-3
            for sc in range(1, NQ):
                for h in range(2):
                    ps = psP.tile([128, 512], f32, tag="proj", name="proj_ps")
                    for dt_ in range(ND):
                        nc.tensor.matmul(
                            ps[:],
                            wq_sb[:, dt_ * DPC + h * 128: dt_ * DPC + h * 128 + 128],
                            xt_tiles[sc][:, dt_ * 512:(dt_ + 1) * 512],
                            start=(dt_ == 0),
                            stop=(dt_ == ND - 1),
                        )
                    nc.any.tensor_copy(qt_sb[h][:, sc * 512:(sc + 1) * 512], ps[:])

            # ---- phase D: qc=0 P@V, then steady state qc=1..3 ----
            normalize(0, 0, pv(0, 0, pts00))
            normalize(1, 0, pv(1, 0, pts10))

            for qc in range(1, NQ):
                pts, acc = attn_scores(0, qc)
                den_finalize(0, qc, acc)
                normalize(0, qc, pv(0, qc, pts))
                emit_oproj(qc - 1)
                pts, acc = attn_scores(1, qc)
                den_finalize(1, qc, acc)
                normalize(1, qc, pv(1, qc, pts))
            emit_oproj(NQ - 1)

    nc.compile()
    return nc


def _numpy_fallback(x, mask, Wq, bq, Wk, bk, Wv, bv, Wo, bo):
    B, S_, D_ = x.shape
    xf = x.reshape(S_, D_).astype(np.float64)

    def proj(W, b):
        y = xf @ W.astype(np.float64) + b.astype(np.float64)
        return y.reshape(S_, H, DK).transpose(1, 0, 2)

    Q = proj(Wq, bq)
    K = proj(Wk, bk)
    V = proj(Wv, bv)
    m = np.broadcast_to(mask, (B, H, S_, S_))
    out = np.empty((H, S_, DK))
    for h in range(H):
        sc = Q[h] @ K[h].T / np.sqrt(DK)
        sc = np.where(m[0, h], sc, -np.inf)
        sc -= sc.max(axis=-1, keepdims=True)
        e = np.exp(sc)
        p = e / e.sum(axis=-1, keepdims=True)
        out[h] = p @ V[h]
    o = out.transpose(1, 0, 2).reshape(S_, D_)
    res = o @ Wo.astype(np.float64) + bo.astype(np.float64)
    return res.reshape(B, S_, D_).astype(np.float32)


def kernel(x, mask, Wq, bq, Wk, bk, Wv, bv, Wo, bo):
    x = np.asarray(x, dtype=np.float32)
    mask = np.asarray(mask)
    Wq = np.asarray(Wq, dtype=np.float32)
    Wk = np.asarray(Wk, dtype=np.float32)
    Wv = np.asarray(Wv, dtype=np.float32)
    Wo = np.asarray(Wo, dtype=np.float32)
    bq = np.asarray(bq, dtype=np.float32)
    bk = np.asarray(bk, dtype=np.float32)
    bv = np.asarray(bv, dtype=np.float32)
    bo = np.asarray(bo, dtype=np.float32)

    # Off-benchmark shapes/masks/biases: exact numpy fallback.
    # (bk shifts every score row by a constant -> softmax-invariant; bv and bo
    # are affine in the output and folded in on the host; only bq actually
    # changes the attention pattern in a way the device kernel doesn't model.)
    if x.shape != (1, S, D) or not bool(mask.all()) or np.any(bq):
        return _numpy_fallback(x, mask, Wq, bq, Wk, bk, Wv, bv, Wo, bo)

    from concourse.bass_utils import run_bass_kernel_spmd

    if _PROGRAM[0] is None:
        _PROGRAM[0] = _build_program()
    nc = _PROGRAM[0]

    xT = np.ascontiguousarray(x.reshape(S, D).T.astype(np.float16))
    in_maps = []
    for c in range(N_CORES):
        lo, hi = c * DPC, (c + 1) * DPC
        in_maps.append(
            {
                "xT": xT,
                "wq": np.ascontiguousarray(Wq[:, lo:hi].astype(np.float16)),
                "wk": np.ascontiguousarray(Wk[:, lo:hi].astype(np.float16)),
                "wv": np.ascontiguousarray(Wv[:, lo:hi].astype(np.float16)),
                "wo": np.ascontiguousarray(Wo[lo:hi, :].astype(np.float16)),
            }
        )

    res = run_bass_kernel_spmd(nc, in_maps, list(range(N_CORES)), trace=TRACE)
    _LAST_EXEC_NS[0] = res.exec_time_ns
    _LAST_RESULTS[0] = res

    acc = res.results[0]["out"].astype(np.float64)
    for c in range(1, N_CORES):
        acc += res.results[c]["out"]
    # bv contributes (attn rows sum to 1) a constant bv @ Wo; bo is additive.
    acc += (bv.astype(np.float64) @ Wo) + bo
    return acc.astype(np.float32).reshape(1, S, D)


# revision 4
# speedup vs baseline: 1.2532x; 1.1571x over previous
"""Multi-head attention (B=1, S=2048, D=2048, H=16, d_k=128) on 8 Trainium2
NeuronCores via Bass/Tile.

Sharding: tensor-parallel over heads. Each core owns 2 heads: it gets the
column shards of Wq/Wk/Wv and the row shard of Wo for those heads, computes
its partial output projection (in f16), and the host sums the 8 partials
(the all-reduce equivalent) and adds biases.

All matmuls run in fp16 (single-pass, full PE rate; fp32 PSUM accumulation).

Inputs are pre-packed on the host into partition-major SBUF images
([128, ...] with large contiguous per-partition runs) so every DMA
descriptor is 8-16KB instead of 0.5-1KB -- the input stream sustains
~2x the bandwidth and lands in ~25us instead of ~77us.

Schedule: softmax pointwise work (ACT exp, DVE denominator adds) overlaps
projection matmuls instead of running after them:
  A: x streams chunk-major; K^T projection runs chunk-by-chunk behind it,
     with Q^T chunk 0 slotted in, so scores can start ~35us in.
  B: scores+exp+denominator for (h, qc=0) emitted BEFORE the V projection
     and Q chunks 1-3, giving PE ~75us of independent work while ACT/DVE
     chew on the qc=0 softmax.
  C: P@V for qc=0 once V lands, then steady state per q-chunk: scores ->
     wide exp -> f16 denominator adds -> P@V, with the output projection of
     the previous chunk interleaved.

Pointwise cost structure:
  - exp runs on 2-PSUM-bank [128,1024] tiles (halves ACT per-inst overhead)
  - denominator accumulates in f16 wide tiles (DVE 2x mode)
  - output partials are written/DMA'd as f16 in full [128,2048] row blocks
  - cross-partition denominator reduce on (otherwise idle) GPSIMD
"""

import sys

sys.path.insert(0, "/opt/trn_rl_repo")

import numpy as np

S = 2048
D = 2048
H = 16
DK = 128
N_CORES = 8
HEADS_PER_CORE = H // N_CORES  # 2
DPC = HEADS_PER_CORE * DK  # 256, per-core projection width
SCALE = 1.0 / np.sqrt(DK)
ND = D // 128  # 16 d-tiles of the model dim
NS = S // 128  # 16 k-tiles
NQ = S // 512  # 4 q chunks
XCH = ND * 512  # per-partition elems of one x chunk (d-tile-major)

TRACE = False  # test.py flips this to get an NTFF profile + exec time
_LAST_EXEC_NS = [None]
_LAST_RESULTS = [None]

_PROGRAM = [None]


def _build_program():
    from concourse import bacc, bass_isa, mybir
    from concourse.tile import TileContext

    f32 = mybir.dt.float32
    f16 = mybir.dt.float16

    nc = bacc.Bacc()
    # all inputs pre-packed host-side to partition-major [128, ...] images
    xT = nc.declare_dram_parameter("xT", [128, NQ * XCH], f16, isOutput=False)
    wq = nc.declare_dram_parameter("wq", [128, ND * DPC], f16, isOutput=False)
    wk = nc.declare_dram_parameter("wk", [128, ND * DPC], f16, isOutput=False)
    wv = nc.declare_dram_parameter("wv", [128, ND * DPC], f16, isOutput=False)
    wo = nc.declare_dram_parameter("wo", [128, 2 * S], f16, isOutput=False)
    out = nc.declare_dram_parameter("out", [S, D], f16, isOutput=True)

    EXP = mybir.ActivationFunctionType.Exp

    with TileContext(nc) as tc:
        with (
            tc.tile_pool(name="wpool", bufs=1) as wpool,
            tc.tile_pool(name="xpool", bufs=1) as xpool,
            tc.tile_pool(name="qkv", bufs=1) as qkv,
            tc.tile_pool(name="ppool", bufs=18) as ppool,
            tc.tile_pool(name="apool", bufs=2) as apool,
            tc.tile_pool(name="otpool", bufs=6) as otpool,
            tc.tile_pool(name="obpool", bufs=3) as obpool,
            tc.tile_pool(name="psP", bufs=2, space="PSUM") as psP,
            tc.tile_pool(name="psS", bufs=2, space="PSUM") as psS,
            tc.tile_pool(name="psO", bufs=2, space="PSUM") as psO,
        ):
            wq_sb = wpool.tile([128, ND * DPC], f16, tag="wq")
            wk_sb = wpool.tile([128, ND * DPC], f16, tag="wk")
            wv_sb = wpool.tile([128, ND * DPC], f16, tag="wv")
            wo_sb = wpool.tile([128, 2 * S], f16, tag="wo")
            xt = xpool.tile([128, NQ * XCH], f16, tag="xt")

            # per-head Q^T/K^T [128 dk, S] and V natural [128 k, NS*DPC]
            qt_sb = [qkv.tile([128, S], f16, tag=f"qt{h}", name=f"qt{h}") for h in range(2)]
            kt_sb = [qkv.tile([128, S], f16, tag=f"kt{h}", name=f"kt{h}") for h in range(2)]
            v_sb = qkv.tile([128, NS * DPC], f16, tag="v")

            def xsl(sc, dt_, off, width):
                # x chunk sc, d-tile dt_, columns [off, off+width)
                base = sc * XCH + dt_ * 512 + off
                return xt[:, base:base + width]

            # ---- DMA issue (completion order ~ issue order) ----
            nc.sync.dma_start(out=wk_sb[:], in_=wk[:])
            nc.sync.dma_start(out=xt[:, 0:XCH], in_=xT[:, 0:XCH])
            nc.sync.dma_start(out=wq_sb[:], in_=wq[:])
            for sc in range(1, NQ):
                nc.sync.dma_start(
                    out=xt[:, sc * XCH:(sc + 1) * XCH],
                    in_=xT[:, sc * XCH:(sc + 1) * XCH],
                )
            nc.sync.dma_start(out=wv_sb[:], in_=wv[:])
            nc.sync.dma_start(out=wo_sb[:], in_=wo[:])

            def proj_qk(w_sb, dst, h, sc):
                ps = psP.tile([128, 512], f32, tag="proj", name="proj_ps")
                for dt_ in range(ND):
                    nc.tensor.matmul(
                        ps[:],
                        w_sb[:, dt_ * DPC + h * 128: dt_ * DPC + h * 128 + 128],
                        xsl(sc, dt_, 0, 512),
                        start=(dt_ == 0),
                        stop=(dt_ == ND - 1),
                    )
                nc.any.tensor_copy(dst[h][:, sc * 512:(sc + 1) * 512], ps[:])

            # ---- phase A: K^T projection chunk-by-chunk behind the x
            # stream; Q^T chunk 0 slotted in once wq has landed ----
            for sc in range(NQ):
                for h in range(2):
                    proj_qk(wk_sb, kt_sb, h, sc)
                if sc == 1:
                    for h in range(2):
                        proj_qk(wq_sb, qt_sb, h, 0)

            # ---- attention helpers ----
            den_state = {}

            def attn_scores(h, qc):
                qt_slice = qt_sb[h][:, qc * 512:(qc + 1) * 512]
                pts = []
                acc = None
                for g in range(8):
                    st = psS.tile([128, 1024], f32, tag="st", name="st_ps")
                    for j in range(2):
                        kt = 2 * g + j
                        nc.tensor.matmul(
                            st[:, j * 512:(j + 1) * 512],
                            kt_sb[h][:, kt * 128:(kt + 1) * 128],
                            qt_slice,
                            start=True, stop=True,
                        )
                    pt = ppool.tile([128, 1024], f16, tag="pt", name="pt_t")
                    nc.scalar.activation(pt[:], st[:], EXP, scale=float(SCALE))
                    pts.append(pt)
                    if g == 1:
                        acc = apool.tile([128, 1024], f16, tag="acc", name="acc_t")
                        nc.vector.tensor_add(acc[:], pts[0][:], pts[1][:])
                    elif g > 1:
                        nc.vector.tensor_add(acc[:], acc[:], pt[:])
                return pts, acc

            def den_finalize(h, qc, acc):
                # fold the two 512-wide halves, cross-partition reduce on
                # GPSIMD (broadcasts), fast reciprocal
                accf = apool.tile([128, 512], f16, tag="accf", name="accf_t")
                nc.vector.tensor_add(accf[:], acc[:, 0:512], acc[:, 512:1024])
                bc = apool.tile([128, 512], f32, tag="bc", name="bc_t")
                nc.gpsimd.partition_all_reduce(
                    bc[:], accf[:], 128, bass_isa.ReduceOp.add
                )
                rc = apool.tile([128, 512], f32, tag="rc", name="rc_t")
                nc.vector.reciprocal_approx_fast(rc[:], bc[:])
                den_state[(h, qc)] = rc

            def pv(h, qc, pts):
                oT = psO.tile([128, 512], f32, tag="oT", name="oT_ps")
                for kt in range(NS):
                    pt = pts[kt // 2]
                    half = kt % 2
                    nc.tensor.matmul(
                        oT[:],
                        v_sb[:, kt * DPC + h * 128: kt * DPC + h * 128 + 128],
                        pt[:, half * 512:(half + 1) * 512],
                        start=(kt == 0), stop=(kt == NS - 1),
                    )
                return oT

            ot_tiles = {}

            def normalize(h, qc, oT):
                rc = den_state.pop((h, qc))
                ot = otpool.tile([128, 512], f16, tag="ot", name="ot_t")
                nc.vector.tensor_mul(ot[:], oT[:], rc[:])
                ot_tiles[(h, qc)] = ot

            def emit_oproj(qc):
                for qt_ in range(4):
                    ob = obpool.tile([128, 2048], f16, tag="ob", name="ob_t")
                    for ec in range(NQ):
                        ps = psP.tile([128, 512], f32, tag="proj", name="proj_ps")
                        for h in range(2):
                            nc.tensor.matmul(
                                ps[:],
                                ot_tiles[(h, qc)][:, qt_ * 128:(qt_ + 1) * 128],
                                wo_sb[:, h * S + ec * 512: h * S + ec * 512 + 512],
                                start=(h == 0),
                                stop=(h == 1),
                            )
                        nc.any.tensor_copy(ob[:, ec * 512:(ec + 1) * 512], ps[:])
                    # one DMA per 128-row block: full 4KB rows of `out`
                    nc.sync.dma_start(
                        out=out[(qc * 4 + qt_) * 128:(qc * 4 + qt_ + 1) * 128, :],
                        in_=ob[:],
                    )

            # ---- phase B: qc=0 softmax pointwise overlaps V + Q1-3 ----
            pts00, acc00 = attn_scores(0, 0)
            pts10, acc10 = attn_scores(1, 0)
            den_finalize(0, 0, acc00)
            den_finalize(1, 0, acc10)

            # V projection (natural layout), all 16 k-tiles
            for sc in range(NQ):
                for st_ in range(4):
                    s_tile = sc * 4 + st_
                    ps = psP.tile([128, 512], f32, tag="proj", name="proj_ps")
                    for dt_ in range(ND):
                        nc.tensor.matmul(
                            ps[:, 0:DPC],
                            xsl(sc, dt_, st_ * 128, 128),
                            wv_sb[:, dt_ * DPC:(dt_ + 1) * DPC],
                            start=(dt_ == 0),
                            stop=(dt_ == ND - 1),
                        )
                    nc.any.tensor_copy(
                        v_sb[:, s_tile * DPC:(s_tile + 1) * DPC], ps[:, 0:DPC]
                    )
            # Q^T projection, chunks 1-3
            for sc in range(1, NQ):
                for h in range(2):
                    proj_qk(wq_sb, qt_sb, h, sc)

            # ---- phase C: qc=0 P@V, then steady state qc=1..3 ----
            normalize(0, 0, pv(0, 0, pts00))
            normalize(1, 0, pv(1, 0, pts10))

            for qc in range(1, NQ):
                pts, acc = attn_scores(0, qc)
                den_finalize(0, qc, acc)
                normalize(0, qc, pv(0, qc, pts))
                emit_oproj(qc - 1)
                pts, acc = attn_scores(1, qc)
                den_finalize(1, qc, acc)
                normalize(1, qc, pv(1, qc, pts))
            emit_oproj(NQ - 1)

    nc.compile()
    return nc


def _numpy_fallback(x, mask, Wq, bq, Wk, bk, Wv, bv, Wo, bo):
    B, S_, D_ = x.shape
    xf = x.reshape(S_, D_).astype(np.float64)

    def proj(W, b):
        y = xf @ W.astype(np.float64) + b.astype(np.float64)
        return y.reshape(S_, H, DK).transpose(1, 0, 2)

    Q = proj(Wq, bq)
    K = proj(Wk, bk)
    V = proj(Wv, bv)
    m = np.broadcast_to(mask, (B, H, S_, S_))
    out = np.empty((H, S_, DK))
    for h in range(H):
        sc = Q[h] @ K[h].T / np.sqrt(DK)
        sc = np.where(m[0, h], sc, -np.inf)
        sc -= sc.max(axis=-1, keepdims=True)
        e = np.exp(sc)
        p = e / e.sum(axis=-1, keepdims=True)
        out[h] = p @ V[h]
    o = out.transpose(1, 0, 2).reshape(S_, D_)
    res = o @ Wo.astype(np.float64) + bo.astype(np.float64)
    return res.reshape(B, S_, D_).astype(np.float32)


def _pack_x(x):
    # [D, S] -> [128, NQ, ND, 512]: chunk-major, then d-tile, then seq-in-chunk
    xT = x.reshape(S, D).T.astype(np.float16)  # [D, S]
    p = xT.reshape(ND, 128, NQ, 512).transpose(1, 2, 0, 3)
    return np.ascontiguousarray(p.reshape(128, NQ * XCH))


def _pack_w(Wc):
    # [D, DPC] -> [128, ND*DPC]
    p = Wc.astype(np.float16).reshape(ND, 128, DPC).transpose(1, 0, 2)
    return np.ascontiguousarray(p.reshape(128, ND * DPC))


def _pack_wo(Woc):
    # [DPC, S] -> [128, 2*S]
    p = Woc.astype(np.float16).reshape(2, 128, S).transpose(1, 0, 2)
    return np.ascontiguousarray(p.reshape(128, 2 * S))


def kernel(x, mask, Wq, bq, Wk, bk, Wv, bv, Wo, bo):
    x = np.asarray(x, dtype=np.float32)
    mask = np.asarray(mask)
    Wq = np.asarray(Wq, dtype=np.float32)
    Wk = np.asarray(Wk, dtype=np.float32)
    Wv = np.asarray(Wv, dtype=np.float32)
    Wo = np.asarray(Wo, dtype=np.float32)
    bq = np.asarray(bq, dtype=np.float32)
    bk = np.asarray(bk, dtype=np.float32)
    bv = np.asarray(bv, dtype=np.float32)
    bo = np.asarray(bo, dtype=np.float32)

    # Off-benchmark shapes/masks/biases: exact numpy fallback.
    # (bk shifts every score row by a constant -> softmax-invariant; bv and bo
    # are affine in the output and folded in on the host; only bq actually
    # changes the attention pattern in a way the device kernel doesn't model.)
    if x.shape != (1, S, D) or not bool(mask.all()) or np.any(bq):
        return _numpy_fallback(x, mask, Wq, bq, Wk, bk, Wv, bv, Wo, bo)

    from concourse.bass_utils import run_bass_kernel_spmd

    if _PROGRAM[0] is None:
        _PROGRAM[0] = _build_program()
    nc = _PROGRAM[0]

    xp = _pack_x(x)
    in_maps = []
    for c in range(N_CORES):
        lo, hi = c * DPC, (c + 1) * DPC
        in_maps.append(
            {
                "xT": xp,
                "wq": _pack_w(Wq[:, lo:hi]),
                "wk": _pack_w(Wk[:, lo:hi]),
                "wv": _pack_w(Wv[:, lo:hi]),
                "wo": _pack_wo(Wo[lo:hi, :]),
            }
        )

    res = run_bass_kernel_spmd(nc, in_maps, list(range(N_CORES)), trace=TRACE)
    _LAST_EXEC_NS[0] = res.exec_time_ns
    _LAST_RESULTS[0] = res

    acc = res.results[0]["out"].astype(np.float64)
    for c in range(1, N_CORES):
        acc += res.results[c]["out"]
    # bv contributes (attn rows sum to 1) a constant bv @ Wo; bo is additive.
    acc += (bv.astype(np.float64) @ Wo) + bo
    return acc.astype(np.float32).reshape(1, S, D)


# revision 5
# speedup vs baseline: 1.3726x; 1.0953x over previous
"""Multi-head attention (B=1, S=2048, D=2048, H=16, d_k=128) on 8 Trainium2
NeuronCores via Bass/Tile.

Sharding: tensor-parallel over heads. Each core owns 2 heads: it gets the
column shards of Wq/Wk/Wv and the row shard of Wo for those heads, computes
its partial output projection (in f16), and the host sums the 8 partials
(the all-reduce equivalent) and adds biases.

All matmuls run in fp16 (single-pass, full PE rate; fp32 PSUM accumulation).

Inputs are pre-packed on the host into partition-major SBUF images
([128, ...] with large contiguous per-partition runs) so every DMA
descriptor is 8-16KB instead of 0.5-1KB -- the input stream sustains
~2x the bandwidth and lands in ~25us instead of ~77us.

Schedule: softmax pointwise work (ACT exp, DVE denominator adds) overlaps
projection matmuls instead of running after them:
  A: x streams chunk-major; K^T projection runs chunk-by-chunk behind it,
     with Q^T chunk 0 slotted in, so scores can start ~35us in.
  B: scores+exp+denominator for (h, qc=0) emitted BEFORE the V projection
     and Q chunks 1-3, giving PE ~75us of independent work while ACT/DVE
     chew on the qc=0 softmax.
  C: P@V for qc=0 once V lands, then steady state per q-chunk: scores ->
     wide exp -> f16 denominator adds -> P@V, with the output projection of
     the previous chunk interleaved.

Pointwise cost structure:
  - exp runs on 2-PSUM-bank [128,1024] tiles (halves ACT per-inst overhead)
  - denominator accumulates in f16 wide tiles (DVE 2x mode)
  - output partials are written/DMA'd as f16 in full [128,2048] row blocks
  - cross-partition denominator reduce on (otherwise idle) GPSIMD
"""

import sys

sys.path.insert(0, "/opt/trn_rl_repo")

import numpy as np

S = 2048
D = 2048
H = 16
DK = 128
N_CORES = 8
HEADS_PER_CORE = H // N_CORES  # 2
DPC = HEADS_PER_CORE * DK  # 256, per-core projection width
SCALE = 1.0 / np.sqrt(DK)
ND = D // 128  # 16 d-tiles of the model dim
NS = S // 128  # 16 k-tiles
NQ = S // 512  # 4 q chunks
XCH = ND * 512  # per-partition elems of one x chunk (d-tile-major)

TRACE = False  # test.py flips this to get an NTFF profile + exec time
_LAST_EXEC_NS = [None]
_LAST_RESULTS = [None]

_PROGRAM = [None]


def _build_program():
    from concourse import bacc, bass_isa, mybir
    from concourse.tile import TileContext

    f32 = mybir.dt.float32
    f16 = mybir.dt.float16

    nc = bacc.Bacc()
    # all inputs pre-packed host-side to partition-major [128, ...] images
    xT = nc.declare_dram_parameter("xT", [128, NQ * XCH], f16, isOutput=False)
    wq = nc.declare_dram_parameter("wq", [128, ND * DPC], f16, isOutput=False)
    wk = nc.declare_dram_parameter("wk", [128, ND * DPC], f16, isOutput=False)
    wv = nc.declare_dram_parameter("wv", [128, ND * DPC], f16, isOutput=False)
    wo = nc.declare_dram_parameter("wo", [128, 2 * S], f16, isOutput=False)
    out = nc.declare_dram_parameter("out", [S, D], f16, isOutput=True)

    EXP = mybir.ActivationFunctionType.Exp

    with TileContext(nc) as tc:
        with (
            tc.tile_pool(name="wpool", bufs=1) as wpool,
            tc.tile_pool(name="xpool", bufs=1) as xpool,
            tc.tile_pool(name="qkv", bufs=1) as qkv,
            tc.tile_pool(name="ppool", bufs=18) as ppool,
            tc.tile_pool(name="apool", bufs=2) as apool,
            tc.tile_pool(name="otpool", bufs=6) as otpool,
            tc.tile_pool(name="gpool", bufs=8) as gpool,
            tc.tile_pool(name="obpool", bufs=3) as obpool,
            tc.tile_pool(name="psP", bufs=2, space="PSUM") as psP,
            tc.tile_pool(name="psS", bufs=2, space="PSUM") as psS,
            tc.tile_pool(name="psO", bufs=2, space="PSUM") as psO,
        ):
            wq_sb = wpool.tile([128, ND * DPC], f16, tag="wq")
            wk_sb = wpool.tile([128, ND * DPC], f16, tag="wk")
            wv_sb = wpool.tile([128, ND * DPC], f16, tag="wv")
            wo_sb = wpool.tile([128, 2 * S], f16, tag="wo")
            xt = xpool.tile([128, NQ * XCH], f16, tag="xt")

            # per-head Q^T/K^T [128 dk, S] and V natural [128 k, NS*DPC]
            qt_sb = [qkv.tile([128, S], f16, tag=f"qt{h}", name=f"qt{h}") for h in range(2)]
            kt_sb = [qkv.tile([128, S], f16, tag=f"kt{h}", name=f"kt{h}") for h in range(2)]
            v_sb = qkv.tile([128, NS * DPC], f16, tag="v")

            def xsl(sc, dt_, off, width):
                # x chunk sc, d-tile dt_, columns [off, off+width)
                base = sc * XCH + dt_ * 512 + off
                return xt[:, base:base + width]

            # ---- staged DMA issue ----
            # DMA engines round-robin bytes across ALL active transfers, so
            # issuing everything up front makes the first-needed tensors land
            # last-ish. Stage the stream with artificial WAR gates: a tiny
            # DVE op reads the previous stage's last column AND the next
            # stage's first column, so the next stage's DMA (write-after-
            # read) cannot start until the previous stage has landed.
            ones_sb = wpool.tile([128, 128], f16, tag="ones")
            nc.vector.memset(ones_sb[:], 1.0)

            def dma_gate(done_col, next_col):
                g = gpool.tile([128, 1], f16, tag="g", name="g_t")
                nc.vector.tensor_add(g[:], done_col, next_col)

            # S1: wk + x chunk 0
            nc.sync.dma_start(out=wk_sb[:], in_=wk[:])
            nc.sync.dma_start(out=xt[:, 0:XCH], in_=xT[:, 0:XCH])
            # S2 (after x0): wq + x chunk 1
            dma_gate(xt[:, XCH - 1:XCH], xt[:, XCH:XCH + 1])
            dma_gate(xt[:, XCH - 1:XCH], wq_sb[:, 0:1])
            nc.sync.dma_start(out=wq_sb[:], in_=wq[:])
            nc.sync.dma_start(out=xt[:, XCH:2 * XCH], in_=xT[:, XCH:2 * XCH])
            # S3 (after x1): x chunks 2+3
            dma_gate(xt[:, 2 * XCH - 1:2 * XCH], xt[:, 2 * XCH:2 * XCH + 1])
            dma_gate(xt[:, 2 * XCH - 1:2 * XCH], xt[:, 3 * XCH:3 * XCH + 1])
            nc.sync.dma_start(out=xt[:, 2 * XCH:3 * XCH], in_=xT[:, 2 * XCH:3 * XCH])
            nc.sync.dma_start(out=xt[:, 3 * XCH:4 * XCH], in_=xT[:, 3 * XCH:4 * XCH])
            # S4 (after x2): wv + wo
            dma_gate(xt[:, 3 * XCH - 1:3 * XCH], wv_sb[:, 0:1])
            dma_gate(xt[:, 3 * XCH - 1:3 * XCH], wo_sb[:, 0:1])
            nc.sync.dma_start(out=wv_sb[:], in_=wv[:])
            nc.sync.dma_start(out=wo_sb[:], in_=wo[:])

            def proj_qk(w_sb, dst, h, sc):
                ps = psP.tile([128, 512], f32, tag="proj", name="proj_ps")
                for dt_ in range(ND):
                    nc.tensor.matmul(
                        ps[:],
                        w_sb[:, dt_ * DPC + h * 128: dt_ * DPC + h * 128 + 128],
                        xsl(sc, dt_, 0, 512),
                        start=(dt_ == 0),
                        stop=(dt_ == ND - 1),
                    )
                nc.any.tensor_copy(dst[h][:, sc * 512:(sc + 1) * 512], ps[:])

            # ---- phase A: K^T projection chunk-by-chunk behind the x
            # stream; Q^T chunk 0 slotted in once wq has landed ----
            for sc in range(NQ):
                for h in range(2):
                    proj_qk(wk_sb, kt_sb, h, sc)
                if sc == 1:
                    for h in range(2):
                        proj_qk(wq_sb, qt_sb, h, 0)

            # ---- attention helpers ----
            den_state = {}

            def attn_scores(h, qc):
                qt_slice = qt_sb[h][:, qc * 512:(qc + 1) * 512]
                pts = []
                acc = None
                for g in range(8):
                    st = psS.tile([128, 1024], f32, tag="st", name="st_ps")
                    for j in range(2):
                        kt = 2 * g + j
                        nc.tensor.matmul(
                            st[:, j * 512:(j + 1) * 512],
                            kt_sb[h][:, kt * 128:(kt + 1) * 128],
                            qt_slice,
                            start=True, stop=True,
                        )
                    pt = ppool.tile([128, 1024], f16, tag="pt", name="pt_t")
                    nc.scalar.activation(pt[:], st[:], EXP, scale=float(SCALE))
                    pts.append(pt)
                    if g == 1:
                        acc = apool.tile([128, 1024], f16, tag="acc", name="acc_t")
                        nc.vector.tensor_add(acc[:], pts[0][:], pts[1][:])
                    elif g > 1:
                        nc.vector.tensor_add(acc[:], acc[:], pt[:])
                return pts, acc

            def den_finalize(h, qc, acc):
                # cross-partition sum via a ones-stationary matmul: both
                # 512-wide halves of acc accumulate into one PSUM tile whose
                # every partition holds the full denominator (broadcast).
                # ~0.5us of PE instead of a 3.5us GPSIMD reduce on the
                # normalize critical path.
                db = psP.tile([128, 512], f32, tag="proj", name="den_ps")
                nc.tensor.matmul(db[:], ones_sb[:], acc[:, 0:512],
                                 start=True, stop=False)
                nc.tensor.matmul(db[:], ones_sb[:], acc[:, 512:1024],
                                 start=False, stop=True)
                rc = apool.tile([128, 512], f32, tag="rc", name="rc_t")
                nc.vector.reciprocal_approx_fast(rc[:], db[:])
                den_state[(h, qc)] = rc

            def pv(h, qc, pts):
                oT = psO.tile([128, 512], f32, tag="oT", name="oT_ps")
                for kt in range(NS):
                    pt = pts[kt // 2]
                    half = kt % 2
                    nc.tensor.matmul(
                        oT[:],
                        v_sb[:, kt * DPC + h * 128: kt * DPC + h * 128 + 128],
                        pt[:, half * 512:(half + 1) * 512],
                        start=(kt == 0), stop=(kt == NS - 1),
                    )
                return oT

            ot_tiles = {}

            def normalize(h, qc, oT):
                rc = den_state.pop((h, qc))
                ot = otpool.tile([128, 512], f16, tag="ot", name="ot_t")
                nc.vector.tensor_mul(ot[:], oT[:], rc[:])
                ot_tiles[(h, qc)] = ot

            def emit_oproj(qc):
                for qt_ in range(4):
                    ob = obpool.tile([128, 2048], f16, tag="ob", name="ob_t")
                    for ec in range(NQ):
                        ps = psP.tile([128, 512], f32, tag="proj", name="proj_ps")
                        for h in range(2):
                            nc.tensor.matmul(
                                ps[:],
                                ot_tiles[(h, qc)][:, qt_ * 128:(qt_ + 1) * 128],
                                wo_sb[:, h * S + ec * 512: h * S + ec * 512 + 512],
                                start=(h == 0),
                                stop=(h == 1),
                            )
                        nc.any.tensor_copy(ob[:, ec * 512:(ec + 1) * 512], ps[:])
                    # one DMA per 128-row block: full 4KB rows of `out`
                    nc.sync.dma_start(
                        out=out[(qc * 4 + qt_) * 128:(qc * 4 + qt_ + 1) * 128, :],
                        in_=ob[:],
                    )

            # ---- phase B: qc=0 softmax pointwise overlaps V + Q1-3 ----
            pts00, acc00 = attn_scores(0, 0)
            pts10, acc10 = attn_scores(1, 0)

            # V projection (natural layout), all 16 k-tiles
            for sc in range(NQ):
                for st_ in range(4):
                    s_tile = sc * 4 + st_
                    ps = psP.tile([128, 512], f32, tag="proj", name="proj_ps")
                    for dt_ in range(ND):
                        nc.tensor.matmul(
                            ps[:, 0:DPC],
                            xsl(sc, dt_, st_ * 128, 128),
                            wv_sb[:, dt_ * DPC:(dt_ + 1) * DPC],
                            start=(dt_ == 0),
                            stop=(dt_ == ND - 1),
                        )
                    nc.any.tensor_copy(
                        v_sb[:, s_tile * DPC:(s_tile + 1) * DPC], ps[:, 0:DPC]
                    )
            # Q^T projection, chunks 1-3
            for sc in range(1, NQ):
                for h in range(2):
                    proj_qk(wq_sb, qt_sb, h, sc)

            # ---- phase C: qc=0 P@V, then steady state qc=1..3 ----
            den_finalize(0, 0, acc00)
            normalize(0, 0, pv(0, 0, pts00))
            den_finalize(1, 0, acc10)
            normalize(1, 0, pv(1, 0, pts10))

            for qc in range(1, NQ):
                pts, acc = attn_scores(0, qc)
                oT = pv(0, qc, pts)
                den_finalize(0, qc, acc)
                normalize(0, qc, oT)
                emit_oproj(qc - 1)
                pts, acc = attn_scores(1, qc)
                oT = pv(1, qc, pts)
                den_finalize(1, qc, acc)
                normalize(1, qc, oT)
            emit_oproj(NQ - 1)

    nc.compile()
    return nc


def _numpy_fallback(x, mask, Wq, bq, Wk, bk, Wv, bv, Wo, bo):
    B, S_, D_ = x.shape
    xf = x.reshape(S_, D_).astype(np.float64)

    def proj(W, b):
        y = xf @ W.astype(np.float64) + b.astype(np.float64)
        return y.reshape(S_, H, DK).transpose(1, 0, 2)

    Q = proj(Wq, bq)
    K = proj(Wk, bk)
    V = proj(Wv, bv)
    m = np.broadcast_to(mask, (B, H, S_, S_))
    out = np.empty((H, S_, DK))
    for h in range(H):
        sc = Q[h] @ K[h].T / np.sqrt(DK)
        sc = np.where(m[0, h], sc, -np.inf)
        sc -= sc.max(axis=-1, keepdims=True)
        e = np.exp(sc)
        p = e / e.sum(axis=-1, keepdims=True)
        out[h] = p @ V[h]
    o = out.transpose(1, 0, 2).reshape(S_, D_)
    res = o @ Wo.astype(np.float64) + bo.astype(np.float64)
    return res.reshape(B, S_, D_).astype(np.float32)


def _pack_x(x):
    # [D, S] -> [128, NQ, ND, 512]: chunk-major, then d-tile, then seq-in-chunk
    xT = x.reshape(S, D).T.astype(np.float16)  # [D, S]
    p = xT.reshape(ND, 128, NQ, 512).transpose(1, 2, 0, 3)
    return np.ascontiguousarray(p.reshape(128, NQ * XCH))


def _pack_w(Wc):
    # [D, DPC] -> [128, ND*DPC]
    p = Wc.astype(np.float16).reshape(ND, 128, DPC).transpose(1, 0, 2)
    return np.ascontiguousarray(p.reshape(128, ND * DPC))


def _pack_wo(Woc):
    # [DPC, S] -> [128, 2*S]
    p = Woc.astype(np.float16).reshape(2, 128, S).transpose(1, 0, 2)
    return np.ascontiguousarray(p.reshape(128, 2 * S))


def kernel(x, mask, Wq, bq, Wk, bk, Wv, bv, Wo, bo):
    x = np.asarray(x, dtype=np.float32)
    mask = np.asarray(mask)
    Wq = np.asarray(Wq, dtype=np.float32)
    Wk = np.asarray(Wk, dtype=np.float32)
    Wv = np.asarray(Wv, dtype=np.float32)
    Wo = np.asarray(Wo, dtype=np.float32)
    bq = np.asarray(bq, dtype=np.float32)
    bk = np.asarray(bk, dtype=np.float32)
    bv = np.asarray(bv, dtype=np.float32)
    bo = np.asarray(bo, dtype=np.float32)

    # Off-benchmark shapes/masks/biases: exact numpy fallback.
    # (bk shifts every score row by a constant -> softmax-invariant; bv and bo
    # are affine in the output and folded in on the host; only bq actually
    # changes the attention pattern in a way the device kernel doesn't model.)
    if x.shape != (1, S, D) or not bool(mask.all()) or np.any(bq):
        return _numpy_fallback(x, mask, Wq, bq, Wk, bk, Wv, bv, Wo, bo)

    from concourse.bass_utils import run_bass_kernel_spmd

    if _PROGRAM[0] is None:
        _PROGRAM[0] = _build_program()
    nc = _PROGRAM[0]

    xp = _pack_x(x)
    in_maps = []
    for c in range(N_CORES):
        lo, hi = c * DPC, (c + 1) * DPC
        in_maps.append(
            {
                "xT": xp,
                "wq": _pack_w(Wq[:, lo:hi]),
                "wk": _pack_w(Wk[:, lo:hi]),
                "wv": _pack_w(Wv[:, lo:hi]),
                "wo": _pack_wo(Wo[lo:hi, :]),
            }
        )

    res = run_bass_kernel_spmd(nc, in_maps, list(range(N_CORES)), trace=TRACE)
    _LAST_EXEC_NS[0] = res.exec_time_ns
    _LAST_RESULTS[0] = res

    acc = res.results[0]["out"].astype(np.float64)
    for c in range(1, N_CORES):
        acc += res.results[c]["out"]
    # bv contributes (attn rows sum to 1) a constant bv @ Wo; bo is additive.
    acc += (bv.astype(np.float64) @ Wo) + bo
    return acc.astype(np.float32).reshape(1, S, D)


# revision 6
# speedup vs baseline: 1.3825x; 1.0072x over previous
"""Multi-head attention (B=1, S=2048, D=2048, H=16, d_k=128) on 8 Trainium2
NeuronCores via Bass/Tile.

Sharding: tensor-parallel over heads. Each core owns 2 heads: it gets the
column shards of Wq/Wk/Wv and the row shard of Wo for those heads, computes
its partial output projection (in f16), and the host sums the 8 partials
(the all-reduce equivalent) and adds biases.

All matmuls run in fp16 (single-pass, full PE rate; fp32 PSUM accumulation).

Inputs are pre-packed on the host into partition-major SBUF images
([128, ...] with large contiguous per-partition runs) so every DMA
descriptor is 8-16KB instead of 0.5-1KB -- the input stream sustains
~2x the bandwidth and lands in ~25us instead of ~77us.

Schedule: softmax pointwise work (ACT exp, DVE denominator adds) overlaps
projection matmuls instead of running after them:
  A: x streams chunk-major; K^T projection runs chunk-by-chunk behind it,
     with Q^T chunk 0 slotted in, so scores can start ~35us in.
  B: scores+exp+denominator for (h, qc=0) emitted BEFORE the V projection
     and Q chunks 1-3, giving PE ~75us of independent work while ACT/DVE
     chew on the qc=0 softmax.
  C: P@V for qc=0 once V lands, then steady state per q-chunk: scores ->
     wide exp -> f16 denominator adds -> P@V, with the output projection of
     the previous chunk interleaved.

Pointwise cost structure:
  - exp runs on 2-PSUM-bank [128,1024] tiles (halves ACT per-inst overhead)
  - denominator accumulates in f16 wide tiles (DVE 2x mode)
  - output partials are written/DMA'd as f16 in full [128,2048] row blocks
  - cross-partition denominator reduce on (otherwise idle) GPSIMD
"""

import sys

sys.path.insert(0, "/opt/trn_rl_repo")

import numpy as np

S = 2048
D = 2048
H = 16
DK = 128
N_CORES = 8
HEADS_PER_CORE = H // N_CORES  # 2
DPC = HEADS_PER_CORE * DK  # 256, per-core projection width
SCALE = 1.0 / np.sqrt(DK)
ND = D // 128  # 16 d-tiles of the model dim
NS = S // 128  # 16 k-tiles
NQ = S // 512  # 4 q chunks
XCH = ND * 512  # per-partition elems of one x chunk (d-tile-major)

TRACE = False  # test.py flips this to get an NTFF profile + exec time
_LAST_EXEC_NS = [None]
_LAST_RESULTS = [None]

_PROGRAM = [None]


def _build_program():
    from concourse import bacc, bass_isa, mybir
    from concourse.tile import TileContext

    f32 = mybir.dt.float32
    f16 = mybir.dt.float16

    nc = bacc.Bacc()
    # all inputs pre-packed host-side to partition-major [128, ...] images
    xT = nc.declare_dram_parameter("xT", [128, NQ * XCH], f16, isOutput=False)
    wq = nc.declare_dram_parameter("wq", [128, ND * DPC], f16, isOutput=False)
    wk = nc.declare_dram_parameter("wk", [128, ND * DPC], f16, isOutput=False)
    wv = nc.declare_dram_parameter("wv", [128, ND * DPC], f16, isOutput=False)
    wo = nc.declare_dram_parameter("wo", [128, 2 * S], f16, isOutput=False)
    out = nc.declare_dram_parameter("out", [S, D], f16, isOutput=True)

    EXP = mybir.ActivationFunctionType.Exp

    with TileContext(nc) as tc:
        with (
            tc.tile_pool(name="wpool", bufs=1) as wpool,
            tc.tile_pool(name="xpool", bufs=1) as xpool,
            tc.tile_pool(name="qkv", bufs=1) as qkv,
            tc.tile_pool(name="ppool", bufs=26) as ppool,
            tc.tile_pool(name="apool", bufs=2) as apool,
            tc.tile_pool(name="otpool", bufs=6) as otpool,
            tc.tile_pool(name="gpool", bufs=8) as gpool,
            tc.tile_pool(name="obpool", bufs=3) as obpool,
            tc.tile_pool(name="psP", bufs=2, space="PSUM") as psP,
            tc.tile_pool(name="psS", bufs=2, space="PSUM") as psS,
            tc.tile_pool(name="psO", bufs=2, space="PSUM") as psO,
        ):
            wq_sb = wpool.tile([128, ND * DPC], f16, tag="wq")
            wk_sb = wpool.tile([128, ND * DPC], f16, tag="wk")
            wv_sb = wpool.tile([128, ND * DPC], f16, tag="wv")
            wo_sb = wpool.tile([128, 2 * S], f16, tag="wo")
            xt = xpool.tile([128, NQ * XCH], f16, tag="xt")

            # per-head Q^T/K^T [128 dk, S] and V natural [128 k, NS*DPC]
            qt_sb = [qkv.tile([128, S], f16, tag=f"qt{h}", name=f"qt{h}") for h in range(2)]
            kt_sb = [qkv.tile([128, S], f16, tag=f"kt{h}", name=f"kt{h}") for h in range(2)]
            v_sb = qkv.tile([128, NS * DPC], f16, tag="v")

            def xsl(sc, dt_, off, width):
                # x chunk sc, d-tile dt_, columns [off, off+width)
                base = sc * XCH + dt_ * 512 + off
                return xt[:, base:base + width]

            # ---- staged DMA issue ----
            # DMA engines round-robin bytes across ALL active transfers, so
            # issuing everything up front makes the first-needed tensors land
            # last-ish. Stage the stream with artificial WAR gates: a tiny
            # DVE op reads the previous stage's last column AND the next
            # stage's first column, so the next stage's DMA (write-after-
            # read) cannot start until the previous stage has landed.
            ones_sb = wpool.tile([128, 128], f16, tag="ones")
            nc.vector.memset(ones_sb[:], 1.0)

            def dma_gate(done_col, next_col):
                g = gpool.tile([128, 1], f16, tag="g", name="g_t")
                nc.vector.tensor_add(g[:], done_col, next_col)

            # S1: wk + x chunk 0
            nc.sync.dma_start(out=wk_sb[:], in_=wk[:])
            nc.sync.dma_start(out=xt[:, 0:XCH], in_=xT[:, 0:XCH])
            # S2 (after x0): wq + x chunk 1
            dma_gate(xt[:, XCH - 1:XCH], xt[:, XCH:XCH + 1])
            dma_gate(xt[:, XCH - 1:XCH], wq_sb[:, 0:1])
            nc.sync.dma_start(out=wq_sb[:], in_=wq[:])
            nc.sync.dma_start(out=xt[:, XCH:2 * XCH], in_=xT[:, XCH:2 * XCH])
            # S3 (after x1): x chunks 2+3
            dma_gate(xt[:, 2 * XCH - 1:2 * XCH], xt[:, 2 * XCH:2 * XCH + 1])
            dma_gate(xt[:, 2 * XCH - 1:2 * XCH], xt[:, 3 * XCH:3 * XCH + 1])
            nc.sync.dma_start(out=xt[:, 2 * XCH:3 * XCH], in_=xT[:, 2 * XCH:3 * XCH])
            nc.sync.dma_start(out=xt[:, 3 * XCH:4 * XCH], in_=xT[:, 3 * XCH:4 * XCH])
            # S4 (after x2): wv + wo
            dma_gate(xt[:, 3 * XCH - 1:3 * XCH], wv_sb[:, 0:1])
            dma_gate(xt[:, 3 * XCH - 1:3 * XCH], wo_sb[:, 0:1])
            nc.sync.dma_start(out=wv_sb[:], in_=wv[:])
            nc.sync.dma_start(out=wo_sb[:], in_=wo[:])

            def proj_qk(w_sb, dst, h, sc):
                ps = psP.tile([128, 512], f32, tag="proj", name="proj_ps")
                for dt_ in range(ND):
                    nc.tensor.matmul(
                        ps[:],
                        w_sb[:, dt_ * DPC + h * 128: dt_ * DPC + h * 128 + 128],
                        xsl(sc, dt_, 0, 512),
                        start=(dt_ == 0),
                        stop=(dt_ == ND - 1),
                    )
                nc.any.tensor_copy(dst[h][:, sc * 512:(sc + 1) * 512], ps[:])

            # ---- phase A: K^T projection chunk-by-chunk behind the x
            # stream; Q^T chunk 0 slotted in once wq has landed ----
            for sc in range(NQ):
                for h in range(2):
                    proj_qk(wk_sb, kt_sb, h, sc)
                if sc == 1:
                    for h in range(2):
                        proj_qk(wq_sb, qt_sb, h, 0)
                if sc == 2:
                    for h in range(2):
                        proj_qk(wq_sb, qt_sb, h, 1)

            # ---- attention helpers ----
            den_state = {}

            def attn_scores(h, qc):
                qt_slice = qt_sb[h][:, qc * 512:(qc + 1) * 512]
                pts = []
                acc = None
                for g in range(8):
                    st = psS.tile([128, 1024], f32, tag="st", name="st_ps")
                    for j in range(2):
                        kt = 2 * g + j
                        nc.tensor.matmul(
                            st[:, j * 512:(j + 1) * 512],
                            kt_sb[h][:, kt * 128:(kt + 1) * 128],
                            qt_slice,
                            start=True, stop=True,
                        )
                    pt = ppool.tile([128, 1024], f16, tag="pt", name="pt_t")
                    nc.scalar.activation(pt[:], st[:], EXP, scale=float(SCALE))
                    pts.append(pt)
                    if g == 1:
                        acc = apool.tile([128, 1024], f16, tag="acc", name="acc_t", bufs=3)
                        nc.vector.tensor_add(acc[:], pts[0][:], pts[1][:])
                    elif g > 1:
                        nc.vector.tensor_add(acc[:], acc[:], pt[:])
                return pts, acc

            def den_finalize(h, qc, acc):
                # cross-partition sum via a ones-stationary matmul: both
                # 512-wide halves of acc accumulate into one PSUM tile whose
                # every partition holds the full denominator (broadcast).
                # ~0.5us of PE instead of a 3.5us GPSIMD reduce on the
                # normalize critical path.
                db = psP.tile([128, 512], f32, tag="proj", name="den_ps")
                nc.tensor.matmul(db[:], ones_sb[:], acc[:, 0:512],
                                 start=True, stop=False)
                nc.tensor.matmul(db[:], ones_sb[:], acc[:, 512:1024],
                                 start=False, stop=True)
                rc = apool.tile([128, 512], f32, tag="rc", name="rc_t", bufs=3)
                nc.vector.reciprocal_approx_fast(rc[:], db[:])
                den_state[(h, qc)] = rc

            def pv(h, qc, pts):
                oT = psO.tile([128, 512], f32, tag="oT", name="oT_ps")
                for kt in range(NS):
                    pt = pts[kt // 2]
                    half = kt % 2
                    nc.tensor.matmul(
                        oT[:],
                        v_sb[:, kt * DPC + h * 128: kt * DPC + h * 128 + 128],
                        pt[:, half * 512:(half + 1) * 512],
                        start=(kt == 0), stop=(kt == NS - 1),
                    )
                return oT

            ot_tiles = {}

            def normalize(h, qc, oT):
                rc = den_state.pop((h, qc))
                ot = otpool.tile([128, 512], f16, tag="ot", name="ot_t")
                nc.vector.tensor_mul(ot[:], oT[:], rc[:])
                ot_tiles[(h, qc)] = ot

            def emit_oproj(qc):
                for qt_ in range(4):
                    ob = obpool.tile([128, 2048], f16, tag="ob", name="ob_t")
                    for ec in range(NQ):
                        ps = psP.tile([128, 512], f32, tag="proj", name="proj_ps")
                        for h in range(2):
                            nc.tensor.matmul(
                                ps[:],
                                ot_tiles[(h, qc)][:, qt_ * 128:(qt_ + 1) * 128],
                                wo_sb[:, h * S + ec * 512: h * S + ec * 512 + 512],
                                start=(h == 0),
                                stop=(h == 1),
                            )
                        nc.any.tensor_copy(ob[:, ec * 512:(ec + 1) * 512], ps[:])
                    # one DMA per 128-row block: full 4KB rows of `out`
                    nc.sync.dma_start(
                        out=out[(qc * 4 + qt_) * 128:(qc * 4 + qt_ + 1) * 128, :],
                        in_=ob[:],
                    )

            # ---- phase B: qc=0 softmax pointwise overlaps V + Q1-3 ----
            pts00, acc00 = attn_scores(0, 0)
            pts10, acc10 = attn_scores(1, 0)
            pts01, acc01 = attn_scores(0, 1)

            # V projection (natural layout), all 16 k-tiles
            for sc in range(NQ):
                for st_ in range(4):
                    s_tile = sc * 4 + st_
                    ps = psP.tile([128, 512], f32, tag="proj", name="proj_ps")
                    for dt_ in range(ND):
                        nc.tensor.matmul(
                            ps[:, 0:DPC],
                            xsl(sc, dt_, st_ * 128, 128),
                            wv_sb[:, dt_ * DPC:(dt_ + 1) * DPC],
                            start=(dt_ == 0),
                            stop=(dt_ == ND - 1),
                        )
                    nc.any.tensor_copy(
                        v_sb[:, s_tile * DPC:(s_tile + 1) * DPC], ps[:, 0:DPC]
                    )
            # Q^T projection, chunks 2-3
            for sc in range(2, NQ):
                for h in range(2):
                    proj_qk(wq_sb, qt_sb, h, sc)

            # ---- phase C: prefetched P@V, then steady state ----
            den_finalize(0, 0, acc00)
            normalize(0, 0, pv(0, 0, pts00))
            den_finalize(1, 0, acc10)
            normalize(1, 0, pv(1, 0, pts10))
            den_finalize(0, 1, acc01)
            normalize(0, 1, pv(0, 1, pts01))
            emit_oproj(0)

            def steady(h, qc):
                pts, acc = attn_scores(h, qc)
                oT = pv(h, qc, pts)
                den_finalize(h, qc, acc)
                normalize(h, qc, oT)

            steady(1, 1)
            steady(0, 2)
            emit_oproj(1)
            steady(1, 2)
            steady(0, 3)
            emit_oproj(2)
            steady(1, 3)
            emit_oproj(NQ - 1)

    nc.compile()
    return nc


def _numpy_fallback(x, mask, Wq, bq, Wk, bk, Wv, bv, Wo, bo):
    B, S_, D_ = x.shape
    xf = x.reshape(S_, D_).astype(np.float64)

    def proj(W, b):
        y = xf @ W.astype(np.float64) + b.astype(np.float64)
        return y.reshape(S_, H, DK).transpose(1, 0, 2)

    Q = proj(Wq, bq)
    K = proj(Wk, bk)
    V = proj(Wv, bv)
    m = np.broadcast_to(mask, (B, H, S_, S_))
    out = np.empty((H, S_, DK))
    for h in range(H):
        sc = Q[h] @ K[h].T / np.sqrt(DK)
        sc = np.where(m[0, h], sc, -np.inf)
        sc -= sc.max(axis=-1, keepdims=True)
        e = np.exp(sc)
        p = e / e.sum(axis=-1, keepdims=True)
        out[h] = p @ V[h]
    o = out.transpose(1, 0, 2).reshape(S_, D_)
    res = o @ Wo.astype(np.float64) + bo.astype(np.float64)
    return res.reshape(B, S_, D_).astype(np.float32)


def _pack_x(x):
    # [D, S] -> [128, NQ, ND, 512]: chunk-major, then d-tile, then seq-in-chunk
    xT = x.reshape(S, D).T.astype(np.float16)  # [D, S]
    p = xT.reshape(ND, 128, NQ, 512).transpose(1, 2, 0, 3)
    return np.ascontiguousarray(p.reshape(128, NQ * XCH))


def _pack_w(Wc):
    # [D, DPC] -> [128, ND*DPC]
    p = Wc.astype(np.float16).reshape(ND, 128, DPC).transpose(1, 0, 2)
    return np.ascontiguousarray(p.reshape(128, ND * DPC))


def _pack_wo(Woc):
    # [DPC, S] -> [128, 2*S]
    p = Woc.astype(np.float16).reshape(2, 128, S).transpose(1, 0, 2)
    return np.ascontiguousarray(p.reshape(128, 2 * S))


def kernel(x, mask, Wq, bq, Wk, bk, Wv, bv, Wo, bo):
    x = np.asarray(x, dtype=np.float32)
    mask = np.asarray(mask)
    Wq = np.asarray(Wq, dtype=np.float32)
    Wk = np.asarray(Wk, dtype=np.float32)
    Wv = np.asarray(Wv, dtype=np.float32)
    Wo = np.asarray(Wo, dtype=np.float32)
    bq = np.asarray(bq, dtype=np.float32)
    bk = np.asarray(bk, dtype=np.float32)
    bv = np.asarray(bv, dtype=np.float32)
    bo = np.asarray(bo, dtype=np.float32)

    # Off-benchmark shapes/masks/biases: exact numpy fallback.
    # (bk shifts every score row by a constant -> softmax-invariant; bv and bo
    # are affine in the output and folded in on the host; only bq actually
    # changes the attention pattern in a way the device kernel doesn't model.)
    if x.shape != (1, S, D) or not bool(mask.all()) or np.any(bq):
        return _numpy_fallback(x, mask, Wq, bq, Wk, bk, Wv, bv, Wo, bo)

    from concourse.bass_utils import run_bass_kernel_spmd

    if _PROGRAM[0] is None:
        _PROGRAM[0] = _build_program()
    nc = _PROGRAM[0]

    xp = _pack_x(x)
    in_maps = []
    for c in range(N_CORES):
        lo, hi = c * DPC, (c + 1) * DPC
        in_maps.append(
            {
                "xT": xp,
                "wq": _pack_w(Wq[:, lo:hi]),
                "wk": _pack_w(Wk[:, lo:hi]),
                "wv": _pack_w(Wv[:, lo:hi]),
                "wo": _pack_wo(Wo[lo:hi, :]),
            }
        )

    res = run_bass_kernel_spmd(nc, in_maps, list(range(N_CORES)), trace=TRACE)
    _LAST_EXEC_NS[0] = res.exec_time_ns
    _LAST_RESULTS[0] = res

    acc = res.results[0]["out"].astype(np.float64)
    for c in range(1, N_CORES):
        acc += res.results[c]["out"]
    # bv contributes (attn rows sum to 1) a constant bv @ Wo; bo is additive.
    acc += (bv.astype(np.float64) @ Wo) + bo
    return acc.astype(np.float32).reshape(1, S, D)


# revision 7
# speedup vs baseline: 1.3849x; 1.0017x over previous
"""Multi-head attention (B=1, S=2048, D=2048, H=16, d_k=128) on 8 Trainium2
NeuronCores via Bass/Tile.

Sharding: tensor-parallel over heads. Each core owns 2 heads: it gets the
column shards of Wq/Wk/Wv and the row shard of Wo for those heads, computes
its partial output projection (in f16), and the host sums the 8 partials
(the all-reduce equivalent) and adds biases.

All matmuls run in fp16 (single-pass, full PE rate; fp32 PSUM accumulation).

Inputs are pre-packed on the host into partition-major SBUF images
([128, ...] with large contiguous per-partition runs) so every DMA
descriptor is 8-16KB instead of 0.5-1KB -- the input stream sustains
~2x the bandwidth and lands in ~25us instead of ~77us.

Schedule: softmax pointwise work (ACT exp, DVE denominator adds) overlaps
projection matmuls instead of running after them:
  A: x streams chunk-major; K^T projection runs chunk-by-chunk behind it,
     with Q^T chunk 0 slotted in, so scores can start ~35us in.
  B: scores+exp+denominator for (h, qc=0) emitted BEFORE the V projection
     and Q chunks 1-3, giving PE ~75us of independent work while ACT/DVE
     chew on the qc=0 softmax.
  C: P@V for qc=0 once V lands, then steady state per q-chunk: scores ->
     wide exp -> f16 denominator adds -> P@V, with the output projection of
     the previous chunk interleaved.

Pointwise cost structure:
  - exp runs on 2-PSUM-bank [128,1024] tiles (halves ACT per-inst overhead)
  - denominator accumulates in f16 wide tiles (DVE 2x mode)
  - output partials are written/DMA'd as f16 in full [128,2048] row blocks
  - cross-partition denominator reduce on (otherwise idle) GPSIMD
"""

import sys

sys.path.insert(0, "/opt/trn_rl_repo")

import numpy as np

S = 2048
D = 2048
H = 16
DK = 128
N_CORES = 8
HEADS_PER_CORE = H // N_CORES  # 2
DPC = HEADS_PER_CORE * DK  # 256, per-core projection width
SCALE = 1.0 / np.sqrt(DK)
ND = D // 128  # 16 d-tiles of the model dim
NS = S // 128  # 16 k-tiles
NQ = S // 512  # 4 q chunks
XCH = ND * 512  # per-partition elems of one x chunk (d-tile-major)

TRACE = False  # test.py flips this to get an NTFF profile + exec time
_LAST_EXEC_NS = [None]
_LAST_RESULTS = [None]

_PROGRAM = [None]


def _build_program():
    from concourse import bacc, bass_isa, mybir
    from concourse.tile import TileContext

    f32 = mybir.dt.float32
    f16 = mybir.dt.float16

    nc = bacc.Bacc()
    # all inputs pre-packed host-side to partition-major [128, ...] images
    xT = nc.declare_dram_parameter("xT", [128, NQ * XCH], f16, isOutput=False)
    wq = nc.declare_dram_parameter("wq", [128, ND * DPC], f16, isOutput=False)
    wk = nc.declare_dram_parameter("wk", [128, ND * DPC], f16, isOutput=False)
    wv = nc.declare_dram_parameter("wv", [128, ND * DPC], f16, isOutput=False)
    wo = nc.declare_dram_parameter("wo", [128, 2 * S], f16, isOutput=False)
    out = nc.declare_dram_parameter("out", [S, D], f16, isOutput=True)

    EXP = mybir.ActivationFunctionType.Exp

    with TileContext(nc) as tc:
        with (
            tc.tile_pool(name="wpool", bufs=1) as wpool,
            tc.tile_pool(name="xpool", bufs=1) as xpool,
            tc.tile_pool(name="qkv", bufs=1) as qkv,
            tc.tile_pool(name="ppool", bufs=26) as ppool,
            tc.tile_pool(name="apool", bufs=2) as apool,
            tc.tile_pool(name="otpool", bufs=6) as otpool,
            tc.tile_pool(name="gpool", bufs=8) as gpool,
            tc.tile_pool(name="obpool", bufs=3) as obpool,
            tc.tile_pool(name="psP", bufs=2, space="PSUM") as psP,
            tc.tile_pool(name="psS", bufs=2, space="PSUM") as psS,
            tc.tile_pool(name="psO", bufs=2, space="PSUM") as psO,
        ):
            wq_sb = wpool.tile([128, ND * DPC], f16, tag="wq")
            wk_sb = wpool.tile([128, ND * DPC], f16, tag="wk")
            wv_sb = wpool.tile([128, ND * DPC], f16, tag="wv")
            wo_sb = wpool.tile([128, 2 * S], f16, tag="wo")
            xt = xpool.tile([128, NQ * XCH], f16, tag="xt")

            # per-head Q^T/K^T [128 dk, S] and V natural [128 k, NS*DPC]
            qt_sb = [qkv.tile([128, S], f16, tag=f"qt{h}", name=f"qt{h}") for h in range(2)]
            kt_sb = [qkv.tile([128, S], f16, tag=f"kt{h}", name=f"kt{h}") for h in range(2)]
            v_sb = qkv.tile([128, NS * DPC], f16, tag="v")

            def xsl(sc, dt_, off, width):
                # x chunk sc, d-tile dt_, columns [off, off+width)
                base = sc * XCH + dt_ * 512 + off
                return xt[:, base:base + width]

            # ---- staged DMA issue ----
            # DMA engines round-robin bytes across ALL active transfers, so
            # issuing everything up front makes the first-needed tensors land
            # last-ish. Stage the stream with artificial WAR gates: a tiny
            # DVE op reads the previous stage's last column AND the next
            # stage's first column, so the next stage's DMA (write-after-
            # read) cannot start until the previous stage has landed.
            ones_sb = wpool.tile([128, 128], f16, tag="ones")
            nc.vector.memset(ones_sb[:], 1.0)

            def dma_gate(done_col, next_col):
                g = gpool.tile([128, 1], f16, tag="g", name="g_t")
                nc.vector.tensor_add(g[:], done_col, next_col)

            HX = XCH // 2
            # S0: wk + first half of x chunk 0 (d-tiles 0-7)
            nc.sync.dma_start(out=wk_sb[:], in_=wk[:])
            nc.sync.dma_start(out=xt[:, 0:HX], in_=xT[:, 0:HX])
            # S1 (after x0a): x0b + wq
            dma_gate(xt[:, HX - 1:HX], xt[:, HX:HX + 1])
            dma_gate(xt[:, HX - 1:HX], wq_sb[:, 0:1])
            nc.sync.dma_start(out=xt[:, HX:XCH], in_=xT[:, HX:XCH])
            nc.sync.dma_start(out=wq_sb[:], in_=wq[:])
            # S2 (after x0b): x chunk 1
            dma_gate(xt[:, XCH - 1:XCH], xt[:, XCH:XCH + 1])
            nc.sync.dma_start(out=xt[:, XCH:2 * XCH], in_=xT[:, XCH:2 * XCH])
            # S3 (after x1): x chunks 2+3
            dma_gate(xt[:, 2 * XCH - 1:2 * XCH], xt[:, 2 * XCH:2 * XCH + 1])
            dma_gate(xt[:, 2 * XCH - 1:2 * XCH], xt[:, 3 * XCH:3 * XCH + 1])
            nc.sync.dma_start(out=xt[:, 2 * XCH:3 * XCH], in_=xT[:, 2 * XCH:3 * XCH])
            nc.sync.dma_start(out=xt[:, 3 * XCH:4 * XCH], in_=xT[:, 3 * XCH:4 * XCH])
            # S4 (after x2): wv + wo
            dma_gate(xt[:, 3 * XCH - 1:3 * XCH], wv_sb[:, 0:1])
            dma_gate(xt[:, 3 * XCH - 1:3 * XCH], wo_sb[:, 0:1])
            nc.sync.dma_start(out=wv_sb[:], in_=wv[:])
            nc.sync.dma_start(out=wo_sb[:], in_=wo[:])

            def proj_qk(w_sb, dst, h, sc):
                ps = psP.tile([128, 512], f32, tag="proj", name="proj_ps")
                for dt_ in range(ND):
                    nc.tensor.matmul(
                        ps[:],
                        w_sb[:, dt_ * DPC + h * 128: dt_ * DPC + h * 128 + 128],
                        xsl(sc, dt_, 0, 512),
                        start=(dt_ == 0),
                        stop=(dt_ == ND - 1),
                    )
                nc.any.tensor_copy(dst[h][:, sc * 512:(sc + 1) * 512], ps[:])

            # ---- phase A: K^T projection chunk-by-chunk behind the x
            # stream; Q^T chunk 0 slotted in once wq has landed ----
            for sc in range(NQ):
                for h in range(2):
                    proj_qk(wk_sb, kt_sb, h, sc)
                if sc < 2:
                    for h in range(2):
                        proj_qk(wq_sb, qt_sb, h, sc)

            # ---- attention helpers ----
            den_state = {}

            def attn_scores(h, qc):
                qt_slice = qt_sb[h][:, qc * 512:(qc + 1) * 512]
                pts = []
                acc = None
                for g in range(8):
                    st = psS.tile([128, 1024], f32, tag="st", name="st_ps")
                    for j in range(2):
                        kt = 2 * g + j
                        nc.tensor.matmul(
                            st[:, j * 512:(j + 1) * 512],
                            kt_sb[h][:, kt * 128:(kt + 1) * 128],
                            qt_slice,
                            start=True, stop=True,
                        )
                    pt = ppool.tile([128, 1024], f16, tag="pt", name="pt_t")
                    nc.scalar.activation(pt[:], st[:], EXP, scale=float(SCALE))
                    pts.append(pt)
                    if g == 1:
                        acc = apool.tile([128, 1024], f16, tag="acc", name="acc_t", bufs=3)
                        nc.vector.tensor_add(acc[:], pts[0][:], pts[1][:])
                    elif g > 1:
                        nc.vector.tensor_add(acc[:], acc[:], pt[:])
                return pts, acc

            def den_finalize(h, qc, acc):
                # cross-partition sum via a ones-stationary matmul: both
                # 512-wide halves of acc accumulate into one PSUM tile whose
                # every partition holds the full denominator (broadcast).
                # ~0.5us of PE instead of a 3.5us GPSIMD reduce on the
                # normalize critical path.
                db = psP.tile([128, 512], f32, tag="proj", name="den_ps")
                nc.tensor.matmul(db[:], ones_sb[:], acc[:, 0:512],
                                 start=True, stop=False)
                nc.tensor.matmul(db[:], ones_sb[:], acc[:, 512:1024],
                                 start=False, stop=True)
                rc = apool.tile([128, 512], f32, tag="rc", name="rc_t", bufs=3)
                nc.vector.reciprocal_approx_fast(rc[:], db[:])
                den_state[(h, qc)] = rc

            def pv(h, qc, pts):
                oT = psO.tile([128, 512], f32, tag="oT", name="oT_ps")
                for kt in range(NS):
                    pt = pts[kt // 2]
                    half = kt % 2
                    nc.tensor.matmul(
                        oT[:],
                        v_sb[:, kt * DPC + h * 128: kt * DPC + h * 128 + 128],
                        pt[:, half * 512:(half + 1) * 512],
                        start=(kt == 0), stop=(kt == NS - 1),
                    )
                return oT

            ot_tiles = {}

            def normalize(h, qc, oT):
                rc = den_state.pop((h, qc))
                ot = otpool.tile([128, 512], f16, tag="ot", name="ot_t")
                nc.vector.tensor_mul(ot[:], oT[:], rc[:])
                ot_tiles[(h, qc)] = ot

            def emit_oproj(qc):
                for qt_ in range(4):
                    ob = obpool.tile([128, 2048], f16, tag="ob", name="ob_t")
                    for ec in range(NQ):
                        ps = psP.tile([128, 512], f32, tag="proj", name="proj_ps")
                        for h in range(2):
                            nc.tensor.matmul(
                                ps[:],
                                ot_tiles[(h, qc)][:, qt_ * 128:(qt_ + 1) * 128],
                                wo_sb[:, h * S + ec * 512: h * S + ec * 512 + 512],
                                start=(h == 0),
                                stop=(h == 1),
                            )
                        nc.any.tensor_copy(ob[:, ec * 512:(ec + 1) * 512], ps[:])
                    # two DMAs per 128-row block (halves start earlier)
                    r0 = (qc * 4 + qt_) * 128
                    nc.sync.dma_start(out=out[r0:r0 + 128, 0:1024], in_=ob[:, 0:1024])
                    nc.sync.dma_start(out=out[r0:r0 + 128, 1024:2048], in_=ob[:, 1024:2048])

            # ---- phase B: qc=0 softmax pointwise overlaps V + Q1-3 ----
            pts00, acc00 = attn_scores(0, 0)
            pts10, acc10 = attn_scores(1, 0)
            pts01, acc01 = attn_scores(0, 1)

            # V projection (natural layout), all 16 k-tiles
            for sc in range(NQ):
                for st_ in range(4):
                    s_tile = sc * 4 + st_
                    ps = psP.tile([128, 512], f32, tag="proj", name="proj_ps")
                    for dt_ in range(ND):
                        nc.tensor.matmul(
                            ps[:, 0:DPC],
                            xsl(sc, dt_, st_ * 128, 128),
                            wv_sb[:, dt_ * DPC:(dt_ + 1) * DPC],
                            start=(dt_ == 0),
                            stop=(dt_ == ND - 1),
                        )
                    nc.any.tensor_copy(
                        v_sb[:, s_tile * DPC:(s_tile + 1) * DPC], ps[:, 0:DPC]
                    )
            # Q^T projection, chunks 2-3
            for sc in range(2, NQ):
                for h in range(2):
                    proj_qk(wq_sb, qt_sb, h, sc)

            # ---- phase C: prefetched P@V, then steady state ----
            den_finalize(0, 0, acc00)
            normalize(0, 0, pv(0, 0, pts00))
            den_finalize(1, 0, acc10)
            normalize(1, 0, pv(1, 0, pts10))
            den_finalize(0, 1, acc01)
            normalize(0, 1, pv(0, 1, pts01))
            emit_oproj(0)

            def steady(h, qc):
                pts, acc = attn_scores(h, qc)
                oT = pv(h, qc, pts)
                den_finalize(h, qc, acc)
                normalize(h, qc, oT)

            steady(1, 1)
            steady(0, 2)
            emit_oproj(1)
            steady(1, 2)
            steady(0, 3)
            emit_oproj(2)
            steady(1, 3)
            emit_oproj(NQ - 1)

    nc.compile()
    return nc


def _numpy_fallback(x, mask, Wq, bq, Wk, bk, Wv, bv, Wo, bo):
    B, S_, D_ = x.shape
    xf = x.reshape(S_, D_).astype(np.float64)

    def proj(W, b):
        y = xf @ W.astype(np.float64) + b.astype(np.float64)
        return y.reshape(S_, H, DK).transpose(1, 0, 2)

    Q = proj(Wq, bq)
    K = proj(Wk, bk)
    V = proj(Wv, bv)
    m = np.broadcast_to(mask, (B, H, S_, S_))
    out = np.empty((H, S_, DK))
    for h in range(H):
        sc = Q[h] @ K[h].T / np.sqrt(DK)
        sc = np.where(m[0, h], sc, -np.inf)
        sc -= sc.max(axis=-1, keepdims=True)
        e = np.exp(sc)
        p = e / e.sum(axis=-1, keepdims=True)
        out[h] = p @ V[h]
    o = out.transpose(1, 0, 2).reshape(S_, D_)
    res = o @ Wo.astype(np.float64) + bo.astype(np.float64)
    return res.reshape(B, S_, D_).astype(np.float32)


def _pack_x(x):
    # [D, S] -> [128, NQ, ND, 512]: chunk-major, then d-tile, then seq-in-chunk
    xT = x.reshape(S, D).T.astype(np.float16)  # [D, S]
    p = xT.reshape(ND, 128, NQ, 512).transpose(1, 2, 0, 3)
    return np.ascontiguousarray(p.reshape(128, NQ * XCH))


def _pack_w(Wc):
    # [D, DPC] -> [128, ND*DPC]
    p = Wc.astype(np.float16).reshape(ND, 128, DPC).transpose(1, 0, 2)
    return np.ascontiguousarray(p.reshape(128, ND * DPC))


def _pack_wo(Woc):
    # [DPC, S] -> [128, 2*S]
    p = Woc.astype(np.float16).reshape(2, 128, S).transpose(1, 0, 2)
    return np.ascontiguousarray(p.reshape(128, 2 * S))


def kernel(x, mask, Wq, bq, Wk, bk, Wv, bv, Wo, bo):
    x = np.asarray(x, dtype=np.float32)
    mask = np.asarray(mask)
    Wq = np.asarray(Wq, dtype=np.float32)
    Wk = np.asarray(Wk, dtype=np.float32)
    Wv = np.asarray(Wv, dtype=np.float32)
    Wo = np.asarray(Wo, dtype=np.float32)
    bq = np.asarray(bq, dtype=np.float32)
    bk = np.asarray(bk, dtype=np.float32)
    bv = np.asarray(bv, dtype=np.float32)
    bo = np.asarray(bo, dtype=np.float32)

    # Off-benchmark shapes/masks/biases: exact numpy fallback.
    # (bk shifts every score row by a constant -> softmax-invariant; bv and bo
    # are affine in the output and folded in on the host; only bq actually
    # changes the attention pattern in a way the device kernel doesn't model.)
    if x.shape != (1, S, D) or not bool(mask.all()) or np.any(bq):
        return _numpy_fallback(x, mask, Wq, bq, Wk, bk, Wv, bv, Wo, bo)

    from concourse.bass_utils import run_bass_kernel_spmd

    if _PROGRAM[0] is None:
        _PROGRAM[0] = _build_program()
    nc = _PROGRAM[0]

    xp = _pack_x(x)
    in_maps = []
    for c in range(N_CORES):
        lo, hi = c * DPC, (c + 1) * DPC
        in_maps.append(
            {
                "xT": xp,
                "wq": _pack_w(Wq[:, lo:hi]),
                "wk": _pack_w(Wk[:, lo:hi]),
                "wv": _pack_w(Wv[:, lo:hi]),
                "wo": _pack_wo(Wo[lo:hi, :]),
            }
        )

    res = run_bass_kernel_spmd(nc, in_maps, list(range(N_CORES)), trace=TRACE)
    _LAST_EXEC_NS[0] = res.exec_time_ns
    _LAST_RESULTS[0] = res

    acc = res.results[0]["out"].astype(np.float64)
    for c in range(1, N_CORES):
        acc += res.results[c]["out"]
    # bv contributes (attn rows sum to 1) a constant bv @ Wo; bo is additive.
    acc += (bv.astype(np.float64) @ Wo) + bo
    return acc.astype(np.float32).reshape(1, S, D)
